# revision 18
# baseline (speedup 1.0000x reference)
"""Trainium2 Bass kernel for an 11-layer binarized encoder-decoder CNN.

Contract: kernel(**inputs) takes the FULL inputs from setup_inputs()
(x: (8,3,256,256) f32, params: tuple of 11 (w, b, gamma, beta)) and
returns the FULL output (8, 11, 253, 253) f32.

Strategy (pure data parallel, 8 NeuronCores, one image per core):
- All conv layers run on the tensor engine as 9-tap matmul accumulations
  into PSUM (conv-transpose as 4 output-parity classes with 1/2/4 taps).
- Binarized weights / sign activations are exactly representable in bf16,
  and every intermediate conv output is an exact small integer, so all
  binary-layer arithmetic on the PE is EXACT in bf16 with fp32 PSUM.
- Training-mode BN + binary_tanh folds to `sign(y - mean_c)` because
  gamma=1, beta=0 (asserted); conv bias cancels inside BN entirely.
- Channel means need the batch-global sum: each layer does a tiny
  [128,2] AllReduce across the 8 cores (sync-BN), then a vector-engine
  pass computes (raw >= t)*2-1 into the next layer's padded input plane.
- K=64 layers split the image into two halves mapped to PE row-groups
  (0,0)/(64,0) so two tap-streams run concurrently on the 128x128 array;
  layer 0 (K=27) uses a host-built im2col and 4 concurrent row-groups.
"""

import numpy as np
from contextlib import ExitStack

import ml_dtypes
import concourse.bass as bass
import concourse.bacc as bacc
import concourse.mybir as mybir
import concourse.tile as tile
from concourse.bass_utils import run_bass_kernel_spmd

F32 = mybir.dt.float32
BF16 = mybir.dt.bfloat16
AF = mybir.ActivationFunctionType
ALU = mybir.AluOpType

N_CORES = 8

# ---------------------------------------------------------------- geometry --
# Layer table: (kind, stride, act, Cin, Cout, Hin) ; Hout derived.
# plane layout kinds: 'halves' (C=64 on partitions 0-63 / 64-127 by image
# half), 'full' (C=128), 'chunks' (C=256 as two 128-channel planes).

def make_geom(H0=256):
    g = []
    # returns list of layer dicts L0..L10
    def conv_out(H, s):
        return (H + 2 - 3) // s + 1
    sizes = [H0]
    cfg = [('conv', 1, True, 3, 64), ('bin', 1, True, 64, 64), ('bin', 2, True, 64, 128),
           ('bin', 1, True, 128, 128), ('bin', 2, True, 128, 256), ('bin', 1, True, 256, 256),
           ('binT', 2, True, 256, 128), ('bin', 1, True, 128, 128), ('binT', 2, True, 128, 64),
           ('bin', 1, True, 64, 64), ('bin', 1, False, 64, 11)]
    H = H0
    for (kind, s, act, ci, co) in cfg:
        Hin = H
        H = conv_out(H, s) if kind != 'binT' else 2 * H - 1
        g.append(dict(kind=kind, s=s, act=act, Cin=ci, Cout=co, Hin=Hin, Hout=H))
    return g


def plane_layout(C, H):
    """How an H x H C-channel +/-1 plane is stored in the sbuf arena."""
    if C == 64:
        h0 = (H + 1) // 2          # rows in half 0
        rows = h0 + 2              # stored rows per half (1 halo/overlap each side)
        return dict(kind='halves', C=C, H=H, W=H, pitch=H + 2, rows=rows,
                    h0=h0, elems=rows * (H + 2))
    if C == 128:
        return dict(kind='full', C=C, H=H, W=H, pitch=H + 2, rows=H + 2,
                    elems=(H + 2) * (H + 2))
    if C == 256:
        return dict(kind='chunks', C=C, H=H, W=H, pitch=H + 2, rows=H + 2,
                    elems=2 * (H + 2) * (H + 2))
    raise ValueError(C)


# conv-transpose parity-class tap tables: class (a,b) -> list of (ky,kx,di,dj)
def convt_classes():
    out = {}
    for a in (0, 1):
        for b in (0, 1):
            kys = [(1, 0)] if a == 0 else [(0, 0), (2, 1)]
            kxs = [(1, 0)] if b == 0 else [(0, 0), (2, 1)]
            out[(a, b)] = [(ky, kx, di, dj) for (ky, di) in kys for (kx, dj) in kxs]
    return out

CT_CLASSES = convt_classes()


# ------------------------------------------------------------ host weights --

def prep_weights(params, H0=256):
    geom = make_geom(H0)
    ws = {}
    for i, ((kind, *_), p) in enumerate(zip([(g['kind'],) for g in geom], params)):
        pass
    # L0: [27,64] f32 replicated at partition bases 0/32/64/96 -> [128,64]
    w0 = np.asarray(params[0][0], np.float32)           # (64,3,3,3) = (O,I,ky,kx)
    a0 = w0.transpose(2, 3, 1, 0).reshape(27, 64)       # row = (dy*3+dx)*3+cin
    w0sb = np.zeros((128, 64), np.float32)
    for q in range(4):
        w0sb[32 * q:32 * q + 27] = a0
    ws['w0'] = w0sb
    for i in range(1, 11):
        g = geom[i]
        w = np.asarray(params[i][0], np.float32)
        wq = np.where(w >= 0, 1.0, -1.0).astype(np.float32)
        if g['kind'] == 'binT':
            # torch convT weight (I,O,3,3); effective kernel wt[o,i,ky,kx] = wq[i,o,2-ky,2-kx]
            wt = wq.transpose(1, 0, 2, 3)[:, :, ::-1, ::-1]
        else:
            wt = wq                                      # (O,I,ky,kx)
        O, I = wt.shape[0], wt.shape[1]
        arr = wt.transpose(1, 2, 3, 0).reshape(I, 9 * O)  # [cin, tap*O + o]
        if I == 64:
            sb = np.concatenate([arr, arr], axis=0)       # duplicate for row-groups
        elif I == 128:
            sb = arr
        elif I == 256:
            sb = np.concatenate([arr[:128], arr[128:]], axis=1)  # [128, 2*9*O]
        else:
            raise ValueError(I)
        ws[f'w{i}'] = np.ascontiguousarray(sb.astype(ml_dtypes.bfloat16))
    return ws


def prep_im2col(x_img, H0=256):
    """x_img (3,H,H) f32 -> [128, (H/4)*H] f32 im2col, 4 quarter row-groups."""
    H = H0
    xp = np.pad(x_img, ((0, 0), (1, 1), (1, 1)))
    win = np.lib.stride_tricks.sliding_window_view(xp, (3, 3), axis=(1, 2))
    arr = win.transpose(3, 4, 0, 1, 2).reshape(27, H, H)   # row=(dy*3+dx)*3+cin
    q = H // 4
    out = np.zeros((128, q * H), np.float32)
    for k in range(4):
        out[32 * k:32 * k + 27] = arr[:, k * q:(k + 1) * q, :].reshape(27, q * H)
    return out


# ------------------------------------------------------------- the program --

_CACHE = {}

def build_program(H0=256):
    geom = make_geom(H0)
    Hf = geom[10]['Hout']                      # final H (253 for H0=256)
    NPIXF = Hf * Hf

    nc = bacc.Bacc("TRN2", target_bir_lowering=False, debug=False,
                   num_devices=N_CORES)

    # ---- dram I/O
    im2col_d = nc.dram_tensor("im2col0", [128, (H0 // 4) * H0], F32, kind="ExternalInput")
    wd = {0: nc.dram_tensor("w0", [128, 64], F32, kind="ExternalInput")}
    for i in range(1, 11):
        g = geom[i]
        I, O = g['Cin'], g['Cout']
        cols = 9 * O * (2 if I == 256 else 1)
        wd[i] = nc.dram_tensor(f"w{i}", [128, cols], BF16, kind="ExternalInput")
    out_d = nc.dram_tensor("out", [11, NPIXF], F32, kind="ExternalOutput")
    dbg_d = nc.dram_tensor("dbg", [128, 24], F32, kind="ExternalOutput")
    raw0_d = nc.dram_tensor("raw0", [128, (H0 * H0) // 2], F32)   # L0 raw, halves

    AR_ELEMS = max(pl['elems'] for pl in
                   [plane_layout(geom[i]['Cin'], geom[i]['Hin']) for i in range(1, 11)])
    RAW_BYTES = 65536 if H0 == 256 else max(8192, (H0 * H0) // 2 * 2)

    with tile.TileContext(nc) as tc, ExitStack() as ctx:
        wpool = ctx.enter_context(tc.tile_pool(name="w", bufs=1))
        arena_p = ctx.enter_context(tc.tile_pool(name="arena", bufs=1))
        rawp = ctx.enter_context(tc.tile_pool(name="raw", bufs=1))
        stage = ctx.enter_context(tc.tile_pool(name="stage", bufs=2))
        psum = ctx.enter_context(tc.tile_pool(name="psum", bufs=6, space="PSUM"))
        psumd = ctx.enter_context(tc.tile_pool(name="psumd", bufs=1, space="PSUM"))
        small = ctx.enter_context(tc.tile_pool(name="small", bufs=4))
        dram = ctx.enter_context(tc.tile_pool(name="dram", bufs=4, space="DRAM"))

        # ---- persistent tiles (im2col + w0 stream first so L0 starts early)
        arena = arena_p.tile([128, AR_ELEMS], BF16)
        H0q = (H0 // 4) * H0
        im2col_sb0 = rawp.tile([128, H0q], F32, tag="raw", name="im2col_sb0")
        wsb = {0: wpool.tile([128, 64], F32, tag="w0", name="w0sb")}
        nc.sync.dma_start(wsb[0][:], wd[0][:])
        for ch in range(8):
            c0 = (H0q // 8) * ch
            c1 = (H0q // 8) * (ch + 1)
            nc.sync.dma_start(im2col_sb0[:, c0:c1], im2col_d[:, c0:c1])
        for i in range(1, 11):
            shp = wd[i].shape
            wsb[i] = wpool.tile(list(shp), BF16, tag=f"w{i}", name=f"w{i}sb")
            nc.sync.dma_start(wsb[i][:], wd[i][:])
        dbg = wpool.tile([128, 24], F32, tag="dbg")
        # two persistent ping-pong PSUM tiles for PE warm-keeper matmuls:
        # same-engine WAW needs no semaphores, alternating banks avoids
        # same-bank drain/fill overlap.
        dumA = psumd.tile([128, 512], F32, tag="dumA", name="dumA")
        dumB = psumd.tile([128, 512], F32, tag="dumB", name="dumB")

        def warm_pe(n):
            for _k in range(n):
                dp = dumA if _k % 2 == 0 else dumB
                nc.tensor.matmul(dp[:], wsb[1][:, 0:128], wsb[1][:, 0:512],
                                 start=True, stop=True)
        nc.vector.memset(dbg[:], 0.0)

        # ---- dummy collective to warm up the CC path (overlaps input DMAs)
        warm_in = dram.tile([128, 2], F32)
        warm_out = dram.tile([128, 2], F32)
        warm_sb = small.tile([128, 2], F32)
        nc.vector.memset(warm_sb[:], 0.0)
        nc.sync.dma_start(warm_in[:], warm_sb[:])
        nc.gpsimd.collective_compute(
            "AllReduce", ALU.add, replica_groups=[list(range(N_CORES))],
            ins=[warm_in[:].opt()], outs=[warm_out[:].opt()])

        # =================================================================
        # helpers
        # =================================================================
        def plane_view(pl, h_or_c):
            """3-D [Cpart, rows, pitch] AP into the arena for half/chunk."""
            if pl['kind'] == 'halves':
                p0 = 64 * h_or_c
                v = arena[p0:p0 + 64, 0:pl['rows'] * pl['pitch']]
                return v.rearrange("p (r c) -> p r c", c=pl['pitch'])
            if pl['kind'] == 'full':
                v = arena[:, 0:pl['rows'] * pl['pitch']]
                return v.rearrange("p (r c) -> p r c", c=pl['pitch'])
            # chunks
            off = h_or_c * pl['rows'] * pl['pitch']
            v = arena[:, off:off + pl['rows'] * pl['pitch']]
            return v.rearrange("p (r c) -> p r c", c=pl['pitch'])

        def halo_memset(pl):
            """Zero the 1-px halo ring of a plane (and for halves the outer
            halo rows); interior is written by the sign pass."""
            if pl['kind'] in ('full', 'chunks'):
                n = 2 if pl['kind'] == 'chunks' else 1
                for c in range(n):
                    v = plane_view(pl, c)
                    nc.gpsimd.memset(v[:, 0:1, :], 0.0)
                    nc.gpsimd.memset(v[:, pl['rows'] - 1:pl['rows'], :], 0.0)
                    nc.gpsimd.memset(v[:, :, 0:1], 0.0)
                    nc.gpsimd.memset(v[:, :, pl['pitch'] - 1:pl['pitch']], 0.0)
            else:
                v0, v1 = plane_view(pl, 0), plane_view(pl, 1)
                nc.gpsimd.memset(v0[:, 0:1, :], 0.0)                    # top halo
                bh = pl['H'] - pl['h0'] + 1
                nc.gpsimd.memset(v1[:, bh:bh + 1, :], 0.0)  # bottom halo
                for v in (v0, v1):
                    nc.gpsimd.memset(v[:, :, 0:1], 0.0)
                    nc.gpsimd.memset(v[:, :, pl['pitch'] - 1:pl['pitch']], 0.0)

        def cc_allreduce(payload_sb):
            """AllReduce a [128,2] f32 sbuf tile across the 8 cores; returns
            a [128,2] sbuf tile with the global sums."""
            cin = dram.tile([128, 2], F32)
            cout = dram.tile([128, 2], F32)
            nc.sync.dma_start(cin[:], payload_sb[:])
            nc.gpsimd.collective_compute(
                "AllReduce", ALU.add, replica_groups=[list(range(N_CORES))],
                ins=[cin[:].opt()], outs=[cout[:].opt()])
            g = small.tile([128, 2], F32, tag="gsum")
            nc.sync.dma_start(g[:], cout[:])
            return g

        # =================================================================
        # Layer 0: im2col conv (K=27, 4 row-group quarters), fp32
        # =================================================================
        g0 = geom[0]
        H = g0['Hout']; W = H
        q = H // 4                       # rows per quarter
        im2col_sb = im2col_sb0

        rows_per_t = max(1, 512 // W)
        acc0 = small.tile([64, 256], F32, tag="acc")
        n_evac = 0
        hp = (H * H) // 2                # pixels per half
        for r0 in range(0, q, rows_per_t):
            nr = min(rows_per_t, q - r0)
            pt = [psum.tile([64, nr * W], F32, tag="ps", name=f"p0_{_k}") for _k in range(4)]
            for k in range(4):
                nc.tensor.matmul(
                    pt[k][:], wsb[0][32 * k:32 * k + 27, 0:64],
                    im2col_sb[32 * k:32 * k + 27, r0 * W:(r0 + nr) * W],
                    start=True, stop=True,
                    tile_position=(32 * k, 0))
            for k in range(4):
                st = stage.tile([64, rows_per_t * W], F32, tag="stage")
                nc.scalar.activation(st[0:64, 0:nr * W], pt[k][:], AF.Copy,
                                     accum_out=acc0[:, n_evac:n_evac + 1])
                n_evac += 1
                half = k // 2
                off = (k % 2) * q * W + r0 * W
                nc.sync.dma_start(
                    raw0_d[64 * half:64 * half + 64, off:off + nr * W],
                    st[0:64, 0:nr * W])

        warm_pe(110)
        # ---- stats + threshold for L0
        loc = small.tile([128, 2], F32, tag="pay")
        nc.vector.memset(loc[:], 0.0)
        nc.vector.tensor_reduce(loc[0:64, 0:1], acc0[:, 0:n_evac],
                                mybir.AxisListType.X, ALU.add)
        gs = cc_allreduce(loc)
        t0v = small.tile([128, 1], F32, tag="thr")
        nc.vector.tensor_scalar(t0v[0:64, :], gs[0:64, 0:1],
                                1.0 / (N_CORES * H * W), None, ALU.mult)
        nc.sync.dma_start(t0v[64:128, :], t0v[0:64, :])   # dup for half1 partitions
        nc.vector.tensor_copy(dbg[:, 0:1], t0v[:])

        # ---- sign pass L0: raw0 (dram, f32, halves layout) -> plane P1
        pl1 = plane_layout(64, H)
        CH = max(W, (2048 // W) * W)
        for off in range(0, hp, CH):
            n = min(CH, hp - off)
            st = stage.tile([128, 2048], F32, tag="stage")
            nc.sync.dma_start(st[:, 0:n], raw0_d[:, off:off + n])
            s2 = stage.tile([128, 4096], BF16, tag="stage2")
            nc.vector.tensor_scalar(s2[:, 0:n], st[:, 0:n], t0v[:], 2.0,
                                    ALU.is_ge, ALU.mult)
            r0 = off // W; nr = n // W
            for h in (0, 1):
                v = plane_view(pl1, h)
                nc.vector.tensor_scalar(
                    v[:, 1 + r0:1 + r0 + nr, 1:1 + W],
                    s2[64 * h:64 * h + 64, 0:n].rearrange("p (r c) -> p r c", c=W),
                    1.0, None, ALU.subtract)
        halo_memset(pl1)
        # overlap rows between halves (image rows h0-1 and h0)
        fix_overlaps(nc, plane_view, pl1)

        # =================================================================
        # Layers 1..9
        # =================================================================
        for li in range(1, 10):
            g = geom[li]
            pin = plane_layout(g['Cin'], g['Hin'])
            Ho, Wo, Co = g['Hout'], g['Hout'], g['Cout']
            npix = Ho * Wo

            acc = small.tile([128, 256], F32, tag="acc")
            n_evac = 0
            raw_cols = (npix // (2 if Co == 64 else 1)) * (2 if Co == 256 else 1)
            rawt = rawp.tile([128, RAW_BYTES // 2], BF16, tag="raw")

            def evac(pt, dst_ap, cpart):
                nonlocal n_evac
                if isinstance(pt, tuple):
                    nc.scalar.activation(dst_ap, pt[0], AF.Copy)
                    nc.vector.scalar_tensor_tensor(
                        out=dst_ap, in0=pt[1], scalar=1.0, in1=dst_ap,
                        op0=ALU.mult, op1=ALU.add,
                        accum_out=acc[cpart, n_evac:n_evac + 1])
                else:
                    nc.scalar.activation(dst_ap, pt, AF.Copy,
                                         accum_out=acc[cpart, n_evac:n_evac + 1])
                n_evac += 1

            if g['kind'] == 'bin':
                conv_bin(nc, psum, wsb[li], g, pin, plane_view, rawt, evac)
            else:
                conv_t(nc, psum, wsb[li], g, pin, plane_view, rawt, evac)

            warm_pe(110)
            # ---- stats + threshold (halo memsets first: overlap cc latency)
            halo_memset(plane_layout(Co, Ho))
            loc = small.tile([128, 2], F32, tag="pay")
            nc.vector.memset(loc[:], 0.0)
            if Co == 256:
                nc.vector.tensor_reduce(loc[:, 0:1], acc[:, 0:n_evac:2],
                                        mybir.AxisListType.X, ALU.add)
                nc.vector.tensor_reduce(loc[:, 1:2], acc[:, 1:n_evac:2],
                                        mybir.AxisListType.X, ALU.add)
            else:
                nc.vector.tensor_reduce(loc[0:Co, 0:1], acc[0:Co, 0:n_evac],
                                        mybir.AxisListType.X, ALU.add)
            gs = cc_allreduce(loc)
            tv = small.tile([128, 2], F32, tag="thr")
            inv = 1.0 / (N_CORES * npix)
            if Co == 64:
                nc.vector.tensor_scalar(tv[0:64, 0:1], gs[0:64, 0:1], inv, None, ALU.mult)
                nc.sync.dma_start(tv[64:128, 0:1], tv[0:64, 0:1])
            elif Co == 128:
                nc.vector.tensor_scalar(tv[:, 0:1], gs[:, 0:1], inv, None, ALU.mult)
            else:
                nc.vector.tensor_scalar(tv[:, 0:2], gs[:, 0:2], inv, None, ALU.mult)
            nc.vector.tensor_copy(dbg[:, 2 * li:2 * li + 1], tv[:, 0:1])
            nc.vector.tensor_copy(dbg[:, 2 * li + 1:2 * li + 2], gs[:, 0:1])

            # ---- sign pass -> next plane
            pout = plane_layout(Co, Ho)
            sign_pass(nc, stage, rawt, tv, pout, plane_view, Ho, Wo, Co)
            if pout['kind'] == 'halves':
                fix_overlaps(nc, plane_view, pout)

        # =================================================================
        # Layer 10: conv + full BN (no activation) -> output
        # =================================================================
        g = geom[10]
        pin = plane_layout(64, g['Hin'])
        Ho = g['Hout']; Wo = Ho; npix = Ho * Wo
        h0 = (Ho + 1) // 2
        # raw10: quarters of the image on partition bases 0/32/64/96, f32.
        # Quarter boundaries are aligned to evac tiles (split each half's
        # tile list in two) so a tile never straddles a quarter.
        rows_per_t = max(1, 512 // Wo)
        h0in = pin['h0']
        tl = {h: [(y0h + yy, min(rows_per_t, nrh - yy))
                  for yy in range(0, nrh, rows_per_t)]
              for h, (y0h, nrh) in enumerate([(0, h0), (h0, Ho - h0)])}
        qrows = []
        tile_q = {}
        for h in (0, 1):
            n1 = (len(tl[h]) + 1) // 2
            for gi, seg in enumerate([tl[h][:n1], tl[h][n1:]]):
                qi = 2 * h + gi
                qrows.append((seg[0][0], sum(nr for _, nr in seg)) if seg else (0, 0))
                for t in seg:
                    tile_q[t] = qi
        raw10 = rawp.tile([128, RAW_BYTES // 4], F32, tag="raw")
        nc.gpsimd.memset(raw10[:], 0.0)
        acc = small.tile([11, 256], F32, tag="acc")
        accq = small.tile([11, 256], F32, tag="accq")
        n_evac = 0

        nt10 = max(len(tl[0]), len(tl[1]))
        for ti in range(nt10):
            pts = {}
            for h in (0, 1):
                if ti < len(tl[h]):
                    pts[h] = (psum.tile([11, tl[h][ti][1] * Wo], F32, tag="ps",
                                        name=f"pt10_{h}"),) + tl[h][ti]
            for t9 in range(9):
                dy, dx = t9 // 3, t9 % 3
                for h, (pt, y, nr) in pts.items():
                    vin = plane_view(pin, h)
                    buf0 = y + dy if h == 0 else (y + dy - 1) - (h0in - 1)
                    rhs = vin[:, buf0:buf0 + nr, dx:dx + Wo]
                    nc.tensor.matmul(pt[:], wsb[10][64 * h:64 * h + 64,
                                                    t9 * 11:t9 * 11 + 11],
                                     rhs, start=(t9 == 0), stop=(t9 == 8),
                                     tile_position=(64 * h, 0))
            for h, (pt, y, nr) in pts.items():
                qi = tile_q[(y, nr)]
                off = (y - qrows[qi][0]) * Wo
                nc.scalar.activation(
                    raw10[32 * qi:32 * qi + 11, off:off + nr * Wo], pt[:], AF.Copy,
                    accum_out=acc[:, n_evac:n_evac + 1])
                sq = stage.tile([11, 512], F32, tag="sq")
                rsl = raw10[32 * qi:32 * qi + 11, off:off + nr * Wo]
                nc.vector.scalar_tensor_tensor(
                    out=sq[:, 0:nr * Wo], in0=rsl, scalar=1.0, in1=rsl,
                    op0=ALU.mult, op1=ALU.mult,
                    accum_out=accq[:, n_evac:n_evac + 1])
                n_evac += 1

        loc = small.tile([128, 2], F32, tag="pay")
        nc.vector.memset(loc[:], 0.0)
        nc.vector.tensor_reduce(loc[0:11, 0:1], acc[:, 0:n_evac],
                                mybir.AxisListType.X, ALU.add)
        nc.vector.tensor_reduce(loc[0:11, 1:2], accq[:, 0:n_evac],
                                mybir.AxisListType.X, ALU.add)
        gs = cc_allreduce(loc)
        inv = 1.0 / (N_CORES * npix)
        m = small.tile([128, 1], F32, tag="m")
        qm = small.tile([128, 1], F32, tag="qm")
        nc.vector.tensor_scalar(m[0:11, :], gs[0:11, 0:1], inv, None, ALU.mult)
        nc.vector.tensor_scalar(qm[0:11, :], gs[0:11, 1:2], inv, None, ALU.mult)
        var = small.tile([128, 1], F32, tag="var")
        nc.vector.tensor_tensor(var[0:11, :], m[0:11, :], m[0:11, :], ALU.mult)
        nc.vector.tensor_tensor(var[0:11, :], qm[0:11, :], var[0:11, :], ALU.subtract)
        nc.vector.tensor_scalar(var[0:11, :], var[0:11, :], 1e-4, None, ALU.add)
        sd = small.tile([128, 1], F32, tag="sd")
        nc.scalar.activation(sd[0:11, :], var[0:11, :], AF.Sqrt)
        rs = small.tile([128, 4], F32, tag="rs")
        nc.vector.memset(rs[:], 0.0)
        nc.vector.reciprocal(rs[0:11, 0:1], sd[0:11, :])
        # bias = -m*rs ; out = raw*rs + bias
        nc.vector.tensor_tensor(rs[0:11, 1:2], m[0:11, :], rs[0:11, 0:1], ALU.mult)
        nc.vector.tensor_scalar(rs[0:11, 1:2], rs[0:11, 1:2], -1.0, None, ALU.mult)
        nc.vector.tensor_copy(dbg[0:11, 20:21], m[0:11, :])
        nc.vector.tensor_copy(dbg[0:11, 21:22], rs[0:11, 0:1])
        for qi in (1, 2, 3):
            nc.sync.dma_start(rs[32 * qi:32 * qi + 11, 0:2], rs[0:11, 0:2])
        # affine: one chunked op across all 4 partition groups at once
        # (garbage rows have scale=bias=0 and raw10 was memset -> output 0)
        maxcols = max(nrq for _, nrq in qrows) * Wo
        for off in range(0, maxcols, 2048):
            n = min(2048, maxcols - off)
            ot = stage.tile([128, 2048], F32, tag="stage")
            nc.scalar.activation(ot[0:107, 0:n], raw10[0:107, off:off + n],
                                 AF.Identity, bias=rs[0:107, 1:2],
                                 scale=rs[0:107, 0:1])
            for qi in range(4):
                ncols = qrows[qi][1] * Wo
                lo, hi = off, min(off + n, ncols)
                if lo >= hi:
                    continue
                nc.sync.dma_start(
                    out_d[0:11, qrows[qi][0] * Wo + lo: qrows[qi][0] * Wo + hi],
                    ot[32 * qi:32 * qi + 11, lo - off:hi - off])
        nc.sync.dma_start(dbg_d[:], dbg[:])

    nc.compile()
    return nc


# ---------------------------------------------------------- conv emitters --

def conv_bin(nc, psum, wsb, g, pin, plane_view, rawt, evac):
    """Standard 3x3 conv (stride 1 or 2). Emits matmuls + evacs."""
    s, Ci, Co, Ho = g['s'], g['Cin'], g['Cout'], g['Hout']
    Wo = Ho
    rows_per_t = max(1, 512 // Wo)
    if pin['kind'] == 'halves':
        # output half h comes from input half h; interleave for row-groups
        h0o = (Ho + 1) // 2                       # out rows in half 0
        halves = [(0, h0o), (h0o, Ho - h0o)]
        nt = max(len(range(0, hh[1], rows_per_t)) for hh in halves)
        for ti in range(nt):
            pts = {}
            for h, (y0h, nrh) in enumerate(halves):
                yy = ti * rows_per_t
                if yy >= nrh:
                    continue
                nr = min(rows_per_t, nrh - yy)
                pts[h] = (psum.tile([Co, nr * Wo], F32, tag="ps", name=f"pb{h}"), yy, nr)
            for t9 in range(9):
                dy, dx = t9 // 3, t9 % 3
                for h, (pt, yy, nr) in pts.items():
                    y = halves[h][0] + yy
                    vin = plane_view(pin, h)
                    # input buf row of out row y, tap dy: s*y+dy-1 - buf0
                    buf0_img = -1 if h == 0 else s * halves[1][0] - 1
                    br = s * y + dy - 1 - buf0_img
                    rhs = vin[:, br:br + (nr - 1) * s + 1:s,
                              dx:dx + (Wo - 1) * s + 1:s]
                    nc.tensor.matmul(pt[:], wsb[64 * h:64 * h + 64,
                                                t9 * Co:t9 * Co + Co],
                                     rhs, start=(t9 == 0), stop=(t9 == 8),
                                     tile_position=(64 * h, 0))
            for h, (pt, yy, nr) in pts.items():
                y = halves[h][0] + yy
                dst = raw_dst(rawt, Co, Ho, Wo, y, nr)
                evac(pt[:], dst, slice(0, Co))
    else:
        # full or chunks input: split K into 64-row halves on alternating PE
        # row-groups (LDWEIGHTS of one half overlaps the other's matmul);
        # each row-group accumulates into its own PSUM bank, summed at evac.
        kc = 2 if Ci == 256 else 1
        mc = 2 if Co == 256 else 1
        for y in range(0, Ho, rows_per_t):
            nr = min(rows_per_t, Ho - y)
            for mi in range(mc):
                Mo = min(128, Co)
                pA = psum.tile([Mo, nr * Wo], F32, tag="ps", name="pA")
                pB = psum.tile([Mo, nr * Wo], F32, tag="ps", name="pB")
                nmm = [0, 0]
                ntot = 9 * kc
                for ki in range(kc):
                    vin = plane_view(pin, ki if pin['kind'] == 'chunks' else 0)
                    for t9 in range(9):
                        dy, dx = t9 // 3, t9 % 3
                        br = s * y + dy
                        col0 = (ki * 9 + t9) * Co + mi * 128 if Ci == 256 else t9 * Co + mi * 128
                        for u in (0, 1):
                            rg = (2 * ki + t9 + u) % 2 if kc == 2 else (t9 + u) % 2
                            rg = u
                            pt = (pA, pB)[u]
                            rhs = vin[64 * u:64 * u + 64, br:br + (nr - 1) * s + 1:s,
                                      dx:dx + (Wo - 1) * s + 1:s]
                            nc.tensor.matmul(pt[:], wsb[64 * u:64 * u + 64, col0:col0 + Mo],
                                             rhs, start=(nmm[u] == 0),
                                             stop=(nmm[u] == ntot - 1),
                                             tile_position=(64 * u, 0))
                            nmm[u] += 1
                dst = raw_dst(rawt, Co, Ho, Wo, y, nr, mi)
                evac((pA, pB), dst, slice(0, 128) if Co >= 128 else slice(0, Co))


def conv_t(nc, psum, wsb, g, pin, plane_view, rawt, evac):
    """Conv-transpose stride 2 via 4 output parity classes."""
    Ci, Co, Hi, Ho = g['Cin'], g['Cout'], g['Hin'], g['Hout']
    Wo = Ho
    kc = 2 if Ci == 256 else 1
    for (a, b), taps in CT_CLASSES.items():
        ia = (Ho - a + 1) // 2          # class rows
        jb = (Ho - b + 1) // 2          # class cols
        rows_per_t = max(1, 512 // jb)
        for i0 in range(0, ia, rows_per_t):
            nr = min(rows_per_t, ia - i0)
            pA = psum.tile([Co, nr * jb], F32, tag="ps", name="pA")
            pB = psum.tile([Co, nr * jb], F32, tag="ps", name="pB")
            pt2 = (pA, pB)
            nmm = [0, 0]
            ntot = len(taps) * kc
            for ki in range(kc):
                vin = plane_view(pin, ki if pin['kind'] == 'chunks' else 0)
                for (ky, kx, di, dj) in taps:
                    col0 = (ki * 9 + (ky * 3 + kx)) * Co if Ci == 256 else (ky * 3 + kx) * Co
                    for u in (0, 1):
                        rhs = vin[64 * u:64 * u + 64, 1 + i0 + di:1 + i0 + di + nr,
                                  1 + dj:1 + dj + jb]
                        nc.tensor.matmul(pt2[u][:], wsb[64 * u:64 * u + 64, col0:col0 + Co],
                                         rhs, start=(nmm[u] == 0),
                                         stop=(nmm[u] == ntot - 1),
                                         tile_position=(64 * u, 0))
                        nmm[u] += 1
            # evac into raw, strided by parity class
            if Co == 64:
                # halves raw layout; class rows may straddle half boundary
                h0 = (Ho + 1) // 2
                rows = [2 * (i0 + k) + a for k in range(nr)]
                segs = []
                k = 0
                while k < nr:
                    h = 0 if rows[k] < h0 else 1
                    k2 = k
                    while k2 < nr and (0 if rows[k2] < h0 else 1) == h:
                        k2 += 1
                    segs.append((k, k2, h))
                    k = k2
                for (k, k2, h) in segs:
                    y0 = rows[k] - (0 if h == 0 else h0)
                    v = rawt[64 * h:64 * h + 64, 0:((h0 if h == 0 else Ho - h0) * Wo)]
                    v3 = v.rearrange("p (r c) -> p r c", c=Wo)
                    dst = v3[:, y0:y0 + 2 * (k2 - k - 1) + 1:2, b:b + 2 * (jb - 1) + 1:2]
                    evac((pA[0:64, k * jb:k2 * jb].rearrange("p (r c) -> p r c", c=jb),
                          pB[0:64, k * jb:k2 * jb].rearrange("p (r c) -> p r c", c=jb)),
                         dst, slice(0, 64))
            else:
                v3 = rawt[:, 0:Ho * Wo].rearrange("p (r c) -> p r c", c=Wo)
                dst = v3[:, a + 2 * i0:a + 2 * (i0 + nr - 1) + 1:2, b:b + 2 * (jb - 1) + 1:2]
                evac((pA[:].rearrange("p (r c) -> p r c", c=jb),
                      pB[:].rearrange("p (r c) -> p r c", c=jb)), dst, slice(0, Co))


def raw_dst(rawt, Co, Ho, Wo, y, nr, mi=0):
    """AP into the raw tile for output rows y..y+nr (contiguous layout)."""
    if Co == 64:
        h0 = (Ho + 1) // 2
        h = 0 if y < h0 else 1
        y0 = y - (0 if h == 0 else h0)
        return rawt[64 * h:64 * h + 64, y0 * Wo:(y0 + nr) * Wo]
    if Co == 128:
        return rawt[:, y * Wo:(y + nr) * Wo]
    # Co == 256: chunk mi at offset mi*npix
    npix = Ho * Wo
    return rawt[:, mi * npix + y * Wo: mi * npix + (y + nr) * Wo]


def sign_pass(nc, stage, rawt, tv, pout, plane_view, Ho, Wo, Co):
    """(raw >= t)*2-1 -> padded plane interior (bf16)."""
    if pout['kind'] == 'halves':
        h0 = (Ho + 1) // 2
        for h, (r0, nrh) in enumerate([(0, h0), (h0, Ho - h0)]):
            base = 64 * h
            CH = max(Wo, (4096 // Wo) * Wo)
            for off in range(0, nrh * Wo, CH):
                n = min(CH, nrh * Wo - off)
                s2 = stage.tile([128, 4096], BF16, tag="stage2")
                nc.vector.tensor_scalar(s2[base:base + 64, 0:n],
                                        rawt[base:base + 64, off:off + n],
                                        tv[base:base + 64, 0:1], 2.0,
                                        ALU.is_ge, ALU.mult)
                v = plane_view(pout, h)
                rr = off // Wo
                nc.vector.tensor_scalar(
                    v[:, 1 + rr:1 + rr + n // Wo, 1:1 + Wo],
                    s2[base:base + 64, 0:n].rearrange("p (r c) -> p r c", c=Wo),
                    1.0, None, ALU.subtract)
    elif pout['kind'] == 'full':
        CH = max(Wo, (4096 // Wo) * Wo)
        for off in range(0, Ho * Wo, CH):
            n = min(CH, Ho * Wo - off)
            s2 = stage.tile([128, 4096], BF16, tag="stage2")
            nc.vector.tensor_scalar(s2[:, 0:n], rawt[:, off:off + n],
                                    tv[:, 0:1], 2.0, ALU.is_ge, ALU.mult)
            v = plane_view(pout, 0)
            rr = off // Wo
            nc.vector.tensor_scalar(
                v[:, 1 + rr:1 + rr + n // Wo, 1:1 + Wo],
                s2[:, 0:n].rearrange("p (r c) -> p r c", c=Wo),
                1.0, None, ALU.subtract)
    else:   # chunks (Co=256)
        npix = Ho * Wo
        CH = max(Wo, (4096 // Wo) * Wo)
        for c in range(2):
            for off in range(0, npix, CH):
                n = min(CH, npix - off)
                s2 = stage.tile([128, 4096], BF16, tag="stage2")
                nc.vector.tensor_scalar(s2[:, 0:n], rawt[:, c * npix + off:c * npix + off + n],
                                        tv[:, c:c + 1], 2.0, ALU.is_ge, ALU.mult)
                v = plane_view(pout, c)
                rr = off // Wo
                nc.vector.tensor_scalar(
                    v[:, 1 + rr:1 + rr + n // Wo, 1:1 + Wo],
                    s2[:, 0:n].rearrange("p (r c) -> p r c", c=Wo),
                    1.0, None, ALU.subtract)


def fix_overlaps(nc, plane_view, pl):
    """For half-split planes copy the two boundary rows into the opposite
    half's halo positions (cross-partition, so via DMA)."""
    h0, pitch, rows = pl['h0'], pl['pitch'], pl['rows']
    v0, v1 = plane_view(pl, 0), plane_view(pl, 1)
    # image row h0-1: primary = half0 buf row h0 ; -> half1 buf row 0
    nc.sync.dma_start(v1[:, 0:1, :], v0[:, h0:h0 + 1, :])
    # image row h0: primary = half1 buf row 1 ; -> half0 buf row h0+1
    nc.sync.dma_start(v0[:, h0 + 1:h0 + 2, :], v1[:, 1:2, :])


# ------------------------------------------------------------------ driver --

def kernel(x, params):
    x = np.asarray(x, np.float32)
    for (w, b, gmm, bt) in params:
        assert np.all(np.asarray(gmm) == 1.0) and np.all(np.asarray(bt) == 0.0), \
            "kernel assumes gamma=1, beta=0"
    H0 = x.shape[2]
    if 'nc' not in _CACHE:
        _CACHE['nc'] = build_program(H0)
    nc = _CACHE['nc']
    ws = prep_weights(params, H0)
    in_maps = []
    for i in range(N_CORES):
        m = {'im2col0': prep_im2col(x[i], H0)}
        m.update(ws)
        in_maps.append(m)
    res = run_bass_kernel_spmd(nc, in_maps, core_ids=list(range(N_CORES)))
    _CACHE['last_result'] = res
    Hf = make_geom(H0)[10]['Hout']
    out = np.stack([res.results[i]['out'].reshape(11, Hf, Hf)
                    for i in range(N_CORES)])
    return out.astype(np.float32)


if __name__ == "__main__":
    import pickle, time
    x = np.load('/root/problem/x.npy')
    params = pickle.load(open('/root/problem/params.pkl', 'rb'))
    ref = np.load('/root/problem/ref_out.npy')
    t0 = time.time()
    out = kernel(x, params)
    print("kernel() wall", time.time() - t0)
    err = np.abs(out - ref)
    print("abs max err", err.max(),
          "rel l2", np.linalg.norm(out - ref) / np.linalg.norm(ref),
          "bad pixels", (err > 1e-3).sum())


# revision 23
# speedup vs baseline: 1.0556x; 1.0556x over previous
"""Trainium2 Bass kernel for an 11-layer binarized encoder-decoder CNN.

Contract: kernel(**inputs) takes the FULL inputs from setup_inputs()
(x: (8,3,256,256) f32, params: tuple of 11 (w, b, gamma, beta)) and
returns the FULL output (8, 11, 253, 253) f32.

Strategy (pure data parallel, 8 NeuronCores, one image per core):
- All conv layers run on the tensor engine as 9-tap matmul accumulations
  into PSUM (conv-transpose as 4 output-parity classes with 1/2/4 taps).
- Binarized weights / sign activations are exactly representable in bf16,
  and every intermediate conv output is an exact small integer, so all
  binary-layer arithmetic on the PE is EXACT in bf16 with fp32 PSUM.
- Training-mode BN + binary_tanh folds to `sign(y - mean_c)` because
  gamma=1, beta=0 (asserted); conv bias cancels inside BN entirely.
- Channel means need the batch-global sum: each layer does a tiny
  [128,2] AllReduce across the 8 cores (sync-BN), then a vector-engine
  pass computes (raw >= t)*2-1 into the next layer's padded input plane.
- K=64 layers split the image into two halves mapped to PE row-groups
  (0,0)/(64,0) so two tap-streams run concurrently on the 128x128 array;
  layer 0 (K=27) uses a host-built im2col and 4 concurrent row-groups.
"""

import numpy as np
from contextlib import ExitStack

import ml_dtypes
import concourse.bass as bass
import concourse.bacc as bacc
import concourse.mybir as mybir
import concourse.tile as tile
from concourse.bass_utils import run_bass_kernel_spmd

F32 = mybir.dt.float32
BF16 = mybir.dt.bfloat16
AF = mybir.ActivationFunctionType
ALU = mybir.AluOpType

N_CORES = 8

# ---------------------------------------------------------------- geometry --
# Layer table: (kind, stride, act, Cin, Cout, Hin) ; Hout derived.
# plane layout kinds: 'halves' (C=64 on partitions 0-63 / 64-127 by image
# half), 'full' (C=128), 'chunks' (C=256 as two 128-channel planes).

def make_geom(H0=256):
    g = []
    # returns list of layer dicts L0..L10
    def conv_out(H, s):
        return (H + 2 - 3) // s + 1
    sizes = [H0]
    cfg = [('conv', 1, True, 3, 64), ('bin', 1, True, 64, 64), ('bin', 2, True, 64, 128),
           ('bin', 1, True, 128, 128), ('bin', 2, True, 128, 256), ('bin', 1, True, 256, 256),
           ('binT', 2, True, 256, 128), ('bin', 1, True, 128, 128), ('binT', 2, True, 128, 64),
           ('bin', 1, True, 64, 64), ('bin', 1, False, 64, 11)]
    H = H0
    for (kind, s, act, ci, co) in cfg:
        Hin = H
        H = conv_out(H, s) if kind != 'binT' else 2 * H - 1
        g.append(dict(kind=kind, s=s, act=act, Cin=ci, Cout=co, Hin=Hin, Hout=H))
    return g


def plane_layout(C, H):
    """How an H x H C-channel +/-1 plane is stored in the sbuf arena."""
    if C == 64:
        h0 = (H + 1) // 2          # rows in half 0
        rows = h0 + 2              # stored rows per half (1 halo/overlap each side)
        return dict(kind='halves', C=C, H=H, W=H, pitch=H + 2, rows=rows,
                    h0=h0, elems=rows * (H + 2))
    if C == 128:
        return dict(kind='full', C=C, H=H, W=H, pitch=H + 2, rows=H + 2,
                    elems=(H + 2) * (H + 2))
    if C == 256:
        return dict(kind='chunks', C=C, H=H, W=H, pitch=H + 2, rows=H + 2,
                    elems=2 * (H + 2) * (H + 2))
    raise ValueError(C)


# conv-transpose parity-class tap tables: class (a,b) -> list of (ky,kx,di,dj)
def convt_classes():
    out = {}
    for a in (0, 1):
        for b in (0, 1):
            kys = [(1, 0)] if a == 0 else [(0, 0), (2, 1)]
            kxs = [(1, 0)] if b == 0 else [(0, 0), (2, 1)]
            out[(a, b)] = [(ky, kx, di, dj) for (ky, di) in kys for (kx, dj) in kxs]
    return out

CT_CLASSES = convt_classes()


# ------------------------------------------------------------ host weights --

def prep_weights(params, H0=256):
    geom = make_geom(H0)
    ws = {}
    for i, ((kind, *_), p) in enumerate(zip([(g['kind'],) for g in geom], params)):
        pass
    # L0: [27,64] f32 replicated at partition bases 0/32/64/96 -> [128,64]
    w0 = np.asarray(params[0][0], np.float32)           # (64,3,3,3) = (O,I,ky,kx)
    a0 = w0.transpose(2, 3, 1, 0).reshape(27, 64)       # row = (dy*3+dx)*3+cin
    w0sb = np.zeros((128, 64), np.float32)
    for q in range(4):
        w0sb[32 * q:32 * q + 27] = a0
    ws['w0'] = w0sb
    for i in range(1, 11):
        g = geom[i]
        w = np.asarray(params[i][0], np.float32)
        wq = np.where(w >= 0, 1.0, -1.0).astype(np.float32)
        if g['kind'] == 'binT':
            # torch convT weight (I,O,3,3); effective kernel wt[o,i,ky,kx] = wq[i,o,2-ky,2-kx]
            wt = wq.transpose(1, 0, 2, 3)[:, :, ::-1, ::-1]
        else:
            wt = wq                                      # (O,I,ky,kx)
        O, I = wt.shape[0], wt.shape[1]
        arr = wt.transpose(1, 2, 3, 0).reshape(I, 9 * O)  # [cin, tap*O + o]
        if I == 64:
            sb = np.concatenate([arr, arr], axis=0)       # duplicate for row-groups
        elif I == 128:
            sb = arr
        elif I == 256:
            sb = np.concatenate([arr[:128], arr[128:]], axis=1)  # [128, 2*9*O]
        else:
            raise ValueError(I)
        ws[f'w{i}'] = np.ascontiguousarray(sb.astype(ml_dtypes.bfloat16))
    return ws


def prep_im2col(x_img, H0=256):
    """x_img (3,H,H) f32 -> [128, (H/4)*H] f32 im2col, 4 quarter row-groups."""
    H = H0
    xp = np.pad(x_img, ((0, 0), (1, 1), (1, 1)))
    win = np.lib.stride_tricks.sliding_window_view(xp, (3, 3), axis=(1, 2))
    arr = win.transpose(3, 4, 0, 1, 2).reshape(27, H, H)   # row=(dy*3+dx)*3+cin
    q = H // 4
    out = np.zeros((128, q * H), np.float32)
    for k in range(4):
        out[32 * k:32 * k + 27] = arr[:, k * q:(k + 1) * q, :].reshape(27, q * H)
    return out


# ------------------------------------------------------------- the program --

_CACHE = {}

def build_program(H0=256):
    geom = make_geom(H0)
    Hf = geom[10]['Hout']                      # final H (253 for H0=256)
    NPIXF = Hf * Hf

    nc = bacc.Bacc("TRN2", target_bir_lowering=False, debug=False,
                   num_devices=N_CORES)

    # ---- dram I/O
    im2col_d = nc.dram_tensor("im2col0", [128, (H0 // 4) * H0], F32, kind="ExternalInput")
    wd = {0: nc.dram_tensor("w0", [128, 64], F32, kind="ExternalInput")}
    for i in range(1, 11):
        g = geom[i]
        I, O = g['Cin'], g['Cout']
        cols = 9 * O * (2 if I == 256 else 1)
        wd[i] = nc.dram_tensor(f"w{i}", [128, cols], BF16, kind="ExternalInput")
    out_d = nc.dram_tensor("out", [11, NPIXF], F32, kind="ExternalOutput")
    dbg_d = nc.dram_tensor("dbg", [128, 24], F32, kind="ExternalOutput")
    raw0_d = nc.dram_tensor("raw0", [128, (H0 * H0) // 2], F32)   # L0 raw, halves

    AR_ELEMS = max(pl['elems'] for pl in
                   [plane_layout(geom[i]['Cin'], geom[i]['Hin']) for i in range(1, 11)])
    RAW_BYTES = 65536 if H0 == 256 else max(8192, (H0 * H0) // 2 * 2)

    with tile.TileContext(nc) as tc, ExitStack() as ctx:
        wpool = ctx.enter_context(tc.tile_pool(name="w", bufs=1))
        arena_p = ctx.enter_context(tc.tile_pool(name="arena", bufs=1))
        rawp = ctx.enter_context(tc.tile_pool(name="raw", bufs=1))
        stage = ctx.enter_context(tc.tile_pool(name="stage", bufs=2))
        psum = ctx.enter_context(tc.tile_pool(name="psum", bufs=6, space="PSUM"))
        psumd = ctx.enter_context(tc.tile_pool(name="psumd", bufs=1, space="PSUM"))
        small = ctx.enter_context(tc.tile_pool(name="small", bufs=4))
        dram = ctx.enter_context(tc.tile_pool(name="dram", bufs=4, space="DRAM"))

        # ---- persistent tiles (im2col + w0 stream first so L0 starts early)
        arena = arena_p.tile([128, AR_ELEMS], BF16)
        H0q = (H0 // 4) * H0
        im2col_sb0 = rawp.tile([128, H0q], F32, tag="raw", name="im2col_sb0")
        wsb = {0: wpool.tile([128, 64], F32, tag="w0", name="w0sb")}
        nc.sync.dma_start(wsb[0][:], wd[0][:])
        for ch in range(8):
            c0 = (H0q // 8) * ch
            c1 = (H0q // 8) * (ch + 1)
            nc.sync.dma_start(im2col_sb0[:, c0:c1], im2col_d[:, c0:c1])
        for i in range(1, 11):
            shp = wd[i].shape
            wsb[i] = wpool.tile(list(shp), BF16, tag=f"w{i}", name=f"w{i}sb")
            nc.sync.dma_start(wsb[i][:], wd[i][:])
        dbg = wpool.tile([128, 24], F32, tag="dbg")
        # two persistent ping-pong PSUM tiles for PE warm-keeper matmuls:
        # same-engine WAW needs no semaphores, alternating banks avoids
        # same-bank drain/fill overlap.
        dumA = psumd.tile([128, 512], F32, tag="dumA", name="dumA")
        dumB = psumd.tile([128, 512], F32, tag="dumB", name="dumB")

        def warm_pe(n, anchor):
            """Keep the PE busy through a stats-exchange gap. `anchor` is a
            bf16 [P, >=512] AP written by this layer's evacs: the RAW dep
            pins the dummies to this gap (else the scheduler hoists them)."""
            P = anchor.shape[0]
            N = min(512, anchor.shape[1])
            for _k in range(n):
                dp = dumA if _k % 2 == 0 else dumB
                nc.tensor.matmul(dp[0:128, 0:N], wsb[1][0:P, 0:128],
                                 anchor[0:P, 0:N],
                                 start=True, stop=True)
        nc.vector.memset(dbg[:], 0.0)

        # ---- dummy collective to warm up the CC path (overlaps input DMAs)
        warm_in = dram.tile([128, 2], F32)
        warm_out = dram.tile([128, 2], F32)
        warm_sb = small.tile([128, 2], F32)
        nc.vector.memset(warm_sb[:], 0.0)
        nc.sync.dma_start(warm_in[:], warm_sb[:])
        nc.gpsimd.collective_compute(
            "AllReduce", ALU.add, replica_groups=[list(range(N_CORES))],
            ins=[warm_in[:].opt()], outs=[warm_out[:].opt()])

        # =================================================================
        # helpers
        # =================================================================
        def plane_view(pl, h_or_c):
            """3-D [Cpart, rows, pitch] AP into the arena for half/chunk."""
            if pl['kind'] == 'halves':
                p0 = 64 * h_or_c
                v = arena[p0:p0 + 64, 0:pl['rows'] * pl['pitch']]
                return v.rearrange("p (r c) -> p r c", c=pl['pitch'])
            if pl['kind'] == 'full':
                v = arena[:, 0:pl['rows'] * pl['pitch']]
                return v.rearrange("p (r c) -> p r c", c=pl['pitch'])
            # chunks
            off = h_or_c * pl['rows'] * pl['pitch']
            v = arena[:, off:off + pl['rows'] * pl['pitch']]
            return v.rearrange("p (r c) -> p r c", c=pl['pitch'])

        def halo_memset(pl):
            """Zero the 1-px halo ring of a plane (and for halves the outer
            halo rows); interior is written by the sign pass."""
            if pl['kind'] in ('full', 'chunks'):
                n = 2 if pl['kind'] == 'chunks' else 1
                for c in range(n):
                    v = plane_view(pl, c)
                    nc.gpsimd.memset(v[:, 0:1, :], 0.0)
                    nc.gpsimd.memset(v[:, pl['rows'] - 1:pl['rows'], :], 0.0)
                    nc.gpsimd.memset(v[:, :, 0:1], 0.0)
                    nc.gpsimd.memset(v[:, :, pl['pitch'] - 1:pl['pitch']], 0.0)
            else:
                v0, v1 = plane_view(pl, 0), plane_view(pl, 1)
                nc.gpsimd.memset(v0[:, 0:1, :], 0.0)                    # top halo
                bh = pl['H'] - pl['h0'] + 1
                nc.gpsimd.memset(v1[:, bh:bh + 1, :], 0.0)  # bottom halo
                for v in (v0, v1):
                    nc.gpsimd.memset(v[:, :, 0:1], 0.0)
                    nc.gpsimd.memset(v[:, :, pl['pitch'] - 1:pl['pitch']], 0.0)

        def cc_allreduce(payload_sb):
            """AllReduce a [128,2] f32 sbuf tile across the 8 cores; returns
            a [128,2] sbuf tile with the global sums."""
            cin = dram.tile([128, 2], F32)
            cout = dram.tile([128, 2], F32)
            nc.sync.dma_start(cin[:], payload_sb[:])
            nc.gpsimd.collective_compute(
                "AllReduce", ALU.add, replica_groups=[list(range(N_CORES))],
                ins=[cin[:].opt()], outs=[cout[:].opt()])
            g = small.tile([128, 2], F32, tag="gsum")
            nc.sync.dma_start(g[:], cout[:])
            return g

        # =================================================================
        # Layer 0: im2col conv (K=27, 4 row-group quarters), fp32
        # =================================================================
        g0 = geom[0]
        H = g0['Hout']; W = H
        q = H // 4                       # rows per quarter
        im2col_sb = im2col_sb0

        rows_per_t = max(1, 512 // W)
        acc0 = small.tile([64, 256], F32, tag="acc")
        n_evac = 0
        hp = (H * H) // 2                # pixels per half
        for r0 in range(0, q, rows_per_t):
            nr = min(rows_per_t, q - r0)
            pt = [psum.tile([64, nr * W], F32, tag="ps", name=f"p0_{_k}") for _k in range(4)]
            for k in range(4):
                nc.tensor.matmul(
                    pt[k][:], wsb[0][32 * k:32 * k + 27, 0:64],
                    im2col_sb[32 * k:32 * k + 27, r0 * W:(r0 + nr) * W],
                    start=True, stop=True,
                    tile_position=(32 * k, 0))
            for k in range(4):
                st = stage.tile([64, rows_per_t * W], F32, tag="stage")
                nc.scalar.activation(st[0:64, 0:nr * W], pt[k][:], AF.Copy,
                                     accum_out=acc0[:, n_evac:n_evac + 1])
                n_evac += 1
                half = k // 2
                off = (k % 2) * q * W + r0 * W
                nc.sync.dma_start(
                    raw0_d[64 * half:64 * half + 64, off:off + nr * W],
                    st[0:64, 0:nr * W])

        warm_pe(60, acc0[:, 0:n_evac].bitcast(BF16)[:, 1:2 * n_evac:2])
        # ---- stats + threshold for L0
        loc = small.tile([128, 2], F32, tag="pay")
        nc.vector.memset(loc[:], 0.0)
        nc.vector.tensor_reduce(loc[0:64, 0:1], acc0[:, 0:n_evac],
                                mybir.AxisListType.X, ALU.add)
        gs = cc_allreduce(loc)
        t0v = small.tile([128, 1], F32, tag="thr")
        nc.vector.tensor_scalar(t0v[0:64, :], gs[0:64, 0:1],
                                1.0 / (N_CORES * H * W), None, ALU.mult)
        nc.sync.dma_start(t0v[64:128, :], t0v[0:64, :])   # dup for half1 partitions
        nc.vector.tensor_copy(dbg[:, 0:1], t0v[:])

        # ---- sign pass L0: raw0 (dram, f32, halves layout) -> plane P1
        pl1 = plane_layout(64, H)
        CH = max(W, (2048 // W) * W)
        for off in range(0, hp, CH):
            n = min(CH, hp - off)
            st = stage.tile([128, 2048], F32, tag="stage")
            nc.sync.dma_start(st[:, 0:n], raw0_d[:, off:off + n])
            s2 = stage.tile([128, 4096], BF16, tag="stage2")
            nc.vector.tensor_scalar(s2[:, 0:n], st[:, 0:n], t0v[:], 2.0,
                                    ALU.is_ge, ALU.mult)
            r0 = off // W; nr = n // W
            for h in (0, 1):
                v = plane_view(pl1, h)
                nc.vector.tensor_scalar(
                    v[:, 1 + r0:1 + r0 + nr, 1:1 + W],
                    s2[64 * h:64 * h + 64, 0:n].rearrange("p (r c) -> p r c", c=W),
                    1.0, None, ALU.subtract)
        halo_memset(pl1)
        # overlap rows between halves (image rows h0-1 and h0)
        fix_overlaps(nc, plane_view, pl1)

        # =================================================================
        # Layers 1..9
        # =================================================================
        for li in range(1, 10):
            g = geom[li]
            pin = plane_layout(g['Cin'], g['Hin'])
            Ho, Wo, Co = g['Hout'], g['Hout'], g['Cout']
            npix = Ho * Wo

            acc = small.tile([128, 256], F32, tag="acc")
            n_evac = 0
            raw_cols = (npix // (2 if Co == 64 else 1)) * (2 if Co == 256 else 1)
            rawt = rawp.tile([128, RAW_BYTES // 2], BF16, tag="raw")

            def evac(pt, dst_ap, cpart):
                nonlocal n_evac
                if isinstance(pt, tuple):
                    nc.scalar.activation(dst_ap, pt[0], AF.Copy)
                    nc.vector.scalar_tensor_tensor(
                        out=dst_ap, in0=pt[1], scalar=1.0, in1=dst_ap,
                        op0=ALU.mult, op1=ALU.add,
                        accum_out=acc[cpart, n_evac:n_evac + 1])
                else:
                    nc.scalar.activation(dst_ap, pt, AF.Copy,
                                         accum_out=acc[cpart, n_evac:n_evac + 1])
                n_evac += 1

            if g['kind'] == 'bin':
                conv_bin(nc, psum, wsb[li], g, pin, plane_view, rawt, evac)
            else:
                conv_t(nc, psum, wsb[li], g, pin, plane_view, rawt, evac)

            warm_pe(60, rawt[:, 0:min(512, ((Ho - (Ho + 1) // 2) * Wo) if Co == 64 else npix)])
            # ---- stats + threshold (halo memsets first: overlap cc latency)
            halo_memset(plane_layout(Co, Ho))
            loc = small.tile([128, 2], F32, tag="pay")
            nc.vector.memset(loc[:], 0.0)
            if Co == 256:
                nc.vector.tensor_reduce(loc[:, 0:1], acc[:, 0:n_evac:2],
                                        mybir.AxisListType.X, ALU.add)
                nc.vector.tensor_reduce(loc[:, 1:2], acc[:, 1:n_evac:2],
                                        mybir.AxisListType.X, ALU.add)
            else:
                nc.vector.tensor_reduce(loc[0:Co, 0:1], acc[0:Co, 0:n_evac],
                                        mybir.AxisListType.X, ALU.add)
            gs = cc_allreduce(loc)
            tv = small.tile([128, 2], F32, tag="thr")
            inv = 1.0 / (N_CORES * npix)
            if Co == 64:
                nc.vector.tensor_scalar(tv[0:64, 0:1], gs[0:64, 0:1], inv, None, ALU.mult)
                nc.sync.dma_start(tv[64:128, 0:1], tv[0:64, 0:1])
            elif Co == 128:
                nc.vector.tensor_scalar(tv[:, 0:1], gs[:, 0:1], inv, None, ALU.mult)
            else:
                nc.vector.tensor_scalar(tv[:, 0:2], gs[:, 0:2], inv, None, ALU.mult)
            nc.vector.tensor_copy(dbg[:, 2 * li:2 * li + 1], tv[:, 0:1])
            nc.vector.tensor_copy(dbg[:, 2 * li + 1:2 * li + 2], gs[:, 0:1])

            # ---- sign pass -> next plane
            pout = plane_layout(Co, Ho)
            sign_pass(nc, stage, rawt, tv, pout, plane_view, Ho, Wo, Co)
            if pout['kind'] == 'halves':
                fix_overlaps(nc, plane_view, pout)

        # =================================================================
        # Layer 10: conv + full BN (no activation) -> output
        # =================================================================
        g = geom[10]
        pin = plane_layout(64, g['Hin'])
        Ho = g['Hout']; Wo = Ho; npix = Ho * Wo
        h0 = (Ho + 1) // 2
        # raw10: quarters of the image on partition bases 0/32/64/96, f32.
        # Quarter boundaries are aligned to evac tiles (split each half's
        # tile list in two) so a tile never straddles a quarter.
        rows_per_t = max(1, 512 // Wo)
        h0in = pin['h0']
        tl = {h: [(y0h + yy, min(rows_per_t, nrh - yy))
                  for yy in range(0, nrh, rows_per_t)]
              for h, (y0h, nrh) in enumerate([(0, h0), (h0, Ho - h0)])}
        qrows = []
        tile_q = {}
        for h in (0, 1):
            n1 = (len(tl[h]) + 1) // 2
            for gi, seg in enumerate([tl[h][:n1], tl[h][n1:]]):
                qi = 2 * h + gi
                qrows.append((seg[0][0], sum(nr for _, nr in seg)) if seg else (0, 0))
                for t in seg:
                    tile_q[t] = qi
        raw10 = rawp.tile([128, RAW_BYTES // 4], F32, tag="raw")
        nc.gpsimd.memset(raw10[:], 0.0)
        acc = small.tile([11, 256], F32, tag="acc")
        accq = small.tile([11, 256], F32, tag="accq")
        n_evac = 0

        nt10 = max(len(tl[0]), len(tl[1]))
        for ti in range(nt10):
            pts = {}
            for h in (0, 1):
                if ti < len(tl[h]):
                    pts[h] = (psum.tile([11, tl[h][ti][1] * Wo], F32, tag="ps",
                                        name=f"pt10_{h}"),) + tl[h][ti]
            for t9 in range(9):
                dy, dx = t9 // 3, t9 % 3
                for h, (pt, y, nr) in pts.items():
                    vin = plane_view(pin, h)
                    buf0 = y + dy if h == 0 else (y + dy - 1) - (h0in - 1)
                    rhs = vin[:, buf0:buf0 + nr, dx:dx + Wo]
                    nc.tensor.matmul(pt[:], wsb[10][64 * h:64 * h + 64,
                                                    t9 * 11:t9 * 11 + 11],
                                     rhs, start=(t9 == 0), stop=(t9 == 8),
                                     tile_position=(64 * h, 0))
            for h, (pt, y, nr) in pts.items():
                qi = tile_q[(y, nr)]
                off = (y - qrows[qi][0]) * Wo
                nc.scalar.activation(
                    raw10[32 * qi:32 * qi + 11, off:off + nr * Wo], pt[:], AF.Copy,
                    accum_out=acc[:, n_evac:n_evac + 1])
                sq = stage.tile([11, 512], F32, tag="sq")
                rsl = raw10[32 * qi:32 * qi + 11, off:off + nr * Wo]
                nc.vector.scalar_tensor_tensor(
                    out=sq[:, 0:nr * Wo], in0=rsl, scalar=1.0, in1=rsl,
                    op0=ALU.mult, op1=ALU.mult,
                    accum_out=accq[:, n_evac:n_evac + 1])
                n_evac += 1

        warm_pe(30, raw10[:].bitcast(BF16)[:, 1:1024:2])
        loc = small.tile([128, 2], F32, tag="pay")
        nc.vector.memset(loc[:], 0.0)
        nc.vector.tensor_reduce(loc[0:11, 0:1], acc[:, 0:n_evac],
                                mybir.AxisListType.X, ALU.add)
        nc.vector.tensor_reduce(loc[0:11, 1:2], accq[:, 0:n_evac],
                                mybir.AxisListType.X, ALU.add)
        gs = cc_allreduce(loc)
        inv = 1.0 / (N_CORES * npix)
        m = small.tile([128, 1], F32, tag="m")
        qm = small.tile([128, 1], F32, tag="qm")
        nc.vector.tensor_scalar(m[0:11, :], gs[0:11, 0:1], inv, None, ALU.mult)
        nc.vector.tensor_scalar(qm[0:11, :], gs[0:11, 1:2], inv, None, ALU.mult)
        var = small.tile([128, 1], F32, tag="var")
        nc.vector.tensor_tensor(var[0:11, :], m[0:11, :], m[0:11, :], ALU.mult)
        nc.vector.tensor_tensor(var[0:11, :], qm[0:11, :], var[0:11, :], ALU.subtract)
        nc.vector.tensor_scalar(var[0:11, :], var[0:11, :], 1e-4, None, ALU.add)
        sd = small.tile([128, 1], F32, tag="sd")
        nc.scalar.activation(sd[0:11, :], var[0:11, :], AF.Sqrt)
        rs = small.tile([128, 4], F32, tag="rs")
        nc.vector.memset(rs[:], 0.0)
        nc.vector.reciprocal(rs[0:11, 0:1], sd[0:11, :])
        # bias = -m*rs ; out = raw*rs + bias
        nc.vector.tensor_tensor(rs[0:11, 1:2], m[0:11, :], rs[0:11, 0:1], ALU.mult)
        nc.vector.tensor_scalar(rs[0:11, 1:2], rs[0:11, 1:2], -1.0, None, ALU.mult)
        nc.vector.tensor_copy(dbg[0:11, 20:21], m[0:11, :])
        nc.vector.tensor_copy(dbg[0:11, 21:22], rs[0:11, 0:1])
        for qi in (1, 2, 3):
            nc.sync.dma_start(rs[32 * qi:32 * qi + 11, 0:2], rs[0:11, 0:2])
        # affine: one chunked op across all 4 partition groups at once
        # (garbage rows have scale=bias=0 and raw10 was memset -> output 0)
        maxcols = max(nrq for _, nrq in qrows) * Wo
        for off in range(0, maxcols, 2048):
            n = min(2048, maxcols - off)
            ot = stage.tile([128, 2048], F32, tag="stage")
            nc.scalar.activation(ot[0:107, 0:n], raw10[0:107, off:off + n],
                                 AF.Identity, bias=rs[0:107, 1:2],
                                 scale=rs[0:107, 0:1])
            for qi in range(4):
                ncols = qrows[qi][1] * Wo
                lo, hi = off, min(off + n, ncols)
                if lo >= hi:
                    continue
                nc.sync.dma_start(
                    out_d[0:11, qrows[qi][0] * Wo + lo: qrows[qi][0] * Wo + hi],
                    ot[32 * qi:32 * qi + 11, lo - off:hi - off])
        nc.sync.dma_start(dbg_d[:], dbg[:])

    nc.compile()
    return nc


# ---------------------------------------------------------- conv emitters --

def conv_bin(nc, psum, wsb, g, pin, plane_view, rawt, evac):
    """Standard 3x3 conv (stride 1 or 2). Emits matmuls + evacs."""
    s, Ci, Co, Ho = g['s'], g['Cin'], g['Cout'], g['Hout']
    Wo = Ho
    rows_per_t = max(1, 512 // Wo)
    if pin['kind'] == 'halves':
        # output half h comes from input half h; interleave for row-groups
        h0o = (Ho + 1) // 2                       # out rows in half 0
        halves = [(0, h0o), (h0o, Ho - h0o)]
        nt = max(len(range(0, hh[1], rows_per_t)) for hh in halves)
        for ti in range(nt):
            pts = {}
            for h, (y0h, nrh) in enumerate(halves):
                yy = ti * rows_per_t
                if yy >= nrh:
                    continue
                nr = min(rows_per_t, nrh - yy)
                pts[h] = (psum.tile([Co, nr * Wo], F32, tag="ps", name=f"pb{h}"), yy, nr)
            for t9 in range(9):
                dy, dx = t9 // 3, t9 % 3
                for h, (pt, yy, nr) in pts.items():
                    y = halves[h][0] + yy
                    vin = plane_view(pin, h)
                    # input buf row of out row y, tap dy: s*y+dy-1 - buf0
                    buf0_img = -1 if h == 0 else s * halves[1][0] - 1
                    br = s * y + dy - 1 - buf0_img
                    rhs = vin[:, br:br + (nr - 1) * s + 1:s,
                              dx:dx + (Wo - 1) * s + 1:s]
                    nc.tensor.matmul(pt[:], wsb[64 * h:64 * h + 64,
                                                t9 * Co:t9 * Co + Co],
                                     rhs, start=(t9 == 0), stop=(t9 == 8),
                                     tile_position=(64 * h, 0))
            for h, (pt, yy, nr) in pts.items():
                y = halves[h][0] + yy
                dst = raw_dst(rawt, Co, Ho, Wo, y, nr)
                evac(pt[:], dst, slice(0, Co))
    else:
        # full or chunks input: split K into 64-row halves on alternating PE
        # row-groups (LDWEIGHTS of one half overlaps the other's matmul);
        # each row-group accumulates into its own PSUM bank, summed at evac.
        kc = 2 if Ci == 256 else 1
        mc = 2 if Co == 256 else 1
        for y in range(0, Ho, rows_per_t):
            nr = min(rows_per_t, Ho - y)
            for mi in range(mc):
                Mo = min(128, Co)
                pA = psum.tile([Mo, nr * Wo], F32, tag="ps", name="pA")
                pB = psum.tile([Mo, nr * Wo], F32, tag="ps", name="pB")
                nmm = [0, 0]
                ntot = 9 * kc
                for ki in range(kc):
                    vin = plane_view(pin, ki if pin['kind'] == 'chunks' else 0)
                    for t9 in range(9):
                        dy, dx = t9 // 3, t9 % 3
                        br = s * y + dy
                        col0 = (ki * 9 + t9) * Co + mi * 128 if Ci == 256 else t9 * Co + mi * 128
                        for u in (0, 1):
                            rg = (2 * ki + t9 + u) % 2 if kc == 2 else (t9 + u) % 2
                            rg = u
                            pt = (pA, pB)[u]
                            rhs = vin[64 * u:64 * u + 64, br:br + (nr - 1) * s + 1:s,
                                      dx:dx + (Wo - 1) * s + 1:s]
                            nc.tensor.matmul(pt[:], wsb[64 * u:64 * u + 64, col0:col0 + Mo],
                                             rhs, start=(nmm[u] == 0),
                                             stop=(nmm[u] == ntot - 1),
                                             tile_position=(64 * u, 0))
                            nmm[u] += 1
                dst = raw_dst(rawt, Co, Ho, Wo, y, nr, mi)
                evac((pA, pB), dst, slice(0, 128) if Co >= 128 else slice(0, Co))


def conv_t(nc, psum, wsb, g, pin, plane_view, rawt, evac):
    """Conv-transpose stride 2 via 4 output parity classes."""
    Ci, Co, Hi, Ho = g['Cin'], g['Cout'], g['Hin'], g['Hout']
    Wo = Ho
    kc = 2 if Ci == 256 else 1
    for (a, b), taps in CT_CLASSES.items():
        ia = (Ho - a + 1) // 2          # class rows
        jb = (Ho - b + 1) // 2          # class cols
        rows_per_t = max(1, 512 // jb)
        for i0 in range(0, ia, rows_per_t):
            nr = min(rows_per_t, ia - i0)
            pA = psum.tile([Co, nr * jb], F32, tag="ps", name="pA")
            pB = psum.tile([Co, nr * jb], F32, tag="ps", name="pB")
            pt2 = (pA, pB)
            nmm = [0, 0]
            ntot = len(taps) * kc
            for ki in range(kc):
                vin = plane_view(pin, ki if pin['kind'] == 'chunks' else 0)
                for (ky, kx, di, dj) in taps:
                    col0 = (ki * 9 + (ky * 3 + kx)) * Co if Ci == 256 else (ky * 3 + kx) * Co
                    for u in (0, 1):
                        rhs = vin[64 * u:64 * u + 64, 1 + i0 + di:1 + i0 + di + nr,
                                  1 + dj:1 + dj + jb]
                        nc.tensor.matmul(pt2[u][:], wsb[64 * u:64 * u + 64, col0:col0 + Co],
                                         rhs, start=(nmm[u] == 0),
                                         stop=(nmm[u] == ntot - 1),
                                         tile_position=(64 * u, 0))
                        nmm[u] += 1
            # evac into raw, strided by parity class
            if Co == 64:
                # halves raw layout; class rows may straddle half boundary
                h0 = (Ho + 1) // 2
                rows = [2 * (i0 + k) + a for k in range(nr)]
                segs = []
                k = 0
                while k < nr:
                    h = 0 if rows[k] < h0 else 1
                    k2 = k
                    while k2 < nr and (0 if rows[k2] < h0 else 1) == h:
                        k2 += 1
                    segs.append((k, k2, h))
                    k = k2
                for (k, k2, h) in segs:
                    y0 = rows[k] - (0 if h == 0 else h0)
                    v = rawt[64 * h:64 * h + 64, 0:((h0 if h == 0 else Ho - h0) * Wo)]
                    v3 = v.rearrange("p (r c) -> p r c", c=Wo)
                    dst = v3[:, y0:y0 + 2 * (k2 - k - 1) + 1:2, b:b + 2 * (jb - 1) + 1:2]
                    evac((pA[0:64, k * jb:k2 * jb].rearrange("p (r c) -> p r c", c=jb),
                          pB[0:64, k * jb:k2 * jb].rearrange("p (r c) -> p r c", c=jb)),
                         dst, slice(0, 64))
            else:
                v3 = rawt[:, 0:Ho * Wo].rearrange("p (r c) -> p r c", c=Wo)
                dst = v3[:, a + 2 * i0:a + 2 * (i0 + nr - 1) + 1:2, b:b + 2 * (jb - 1) + 1:2]
                evac((pA[:].rearrange("p (r c) -> p r c", c=jb),
                      pB[:].rearrange("p (r c) -> p r c", c=jb)), dst, slice(0, Co))


def raw_dst(rawt, Co, Ho, Wo, y, nr, mi=0):
    """AP into the raw tile for output rows y..y+nr (contiguous layout)."""
    if Co == 64:
        h0 = (Ho + 1) // 2
        h = 0 if y < h0 else 1
        y0 = y - (0 if h == 0 else h0)
        return rawt[64 * h:64 * h + 64, y0 * Wo:(y0 + nr) * Wo]
    if Co == 128:
        return rawt[:, y * Wo:(y + nr) * Wo]
    # Co == 256: chunk mi at offset mi*npix
    npix = Ho * Wo
    return rawt[:, mi * npix + y * Wo: mi * npix + (y + nr) * Wo]


def sign_pass(nc, stage, rawt, tv, pout, plane_view, Ho, Wo, Co):
    """(raw >= t)*2-1 -> padded plane interior (bf16)."""
    if pout['kind'] == 'halves':
        h0 = (Ho + 1) // 2
        for h, (r0, nrh) in enumerate([(0, h0), (h0, Ho - h0)]):
            base = 64 * h
            CH = max(Wo, (4096 // Wo) * Wo)
            for off in range(0, nrh * Wo, CH):
                n = min(CH, nrh * Wo - off)
                s2 = stage.tile([128, 4096], BF16, tag="stage2")
                nc.vector.tensor_scalar(s2[base:base + 64, 0:n],
                                        rawt[base:base + 64, off:off + n],
                                        tv[base:base + 64, 0:1], 2.0,
                                        ALU.is_ge, ALU.mult)
                v = plane_view(pout, h)
                rr = off // Wo
                nc.vector.tensor_scalar(
                    v[:, 1 + rr:1 + rr + n // Wo, 1:1 + Wo],
                    s2[base:base + 64, 0:n].rearrange("p (r c) -> p r c", c=Wo),
                    1.0, None, ALU.subtract)
    elif pout['kind'] == 'full':
        CH = max(Wo, (4096 // Wo) * Wo)
        for off in range(0, Ho * Wo, CH):
            n = min(CH, Ho * Wo - off)
            s2 = stage.tile([128, 4096], BF16, tag="stage2")
            nc.vector.tensor_scalar(s2[:, 0:n], rawt[:, off:off + n],
                                    tv[:, 0:1], 2.0, ALU.is_ge, ALU.mult)
            v = plane_view(pout, 0)
            rr = off // Wo
            nc.vector.tensor_scalar(
                v[:, 1 + rr:1 + rr + n // Wo, 1:1 + Wo],
                s2[:, 0:n].rearrange("p (r c) -> p r c", c=Wo),
                1.0, None, ALU.subtract)
    else:   # chunks (Co=256)
        npix = Ho * Wo
        CH = max(Wo, (4096 // Wo) * Wo)
        for c in range(2):
            for off in range(0, npix, CH):
                n = min(CH, npix - off)
                s2 = stage.tile([128, 4096], BF16, tag="stage2")
                nc.vector.tensor_scalar(s2[:, 0:n], rawt[:, c * npix + off:c * npix + off + n],
                                        tv[:, c:c + 1], 2.0, ALU.is_ge, ALU.mult)
                v = plane_view(pout, c)
                rr = off // Wo
                nc.vector.tensor_scalar(
                    v[:, 1 + rr:1 + rr + n // Wo, 1:1 + Wo],
                    s2[:, 0:n].rearrange("p (r c) -> p r c", c=Wo),
                    1.0, None, ALU.subtract)


def fix_overlaps(nc, plane_view, pl):
    """For half-split planes copy the two boundary rows into the opposite
    half's halo positions (cross-partition, so via DMA)."""
    h0, pitch, rows = pl['h0'], pl['pitch'], pl['rows']
    v0, v1 = plane_view(pl, 0), plane_view(pl, 1)
    # image row h0-1: primary = half0 buf row h0 ; -> half1 buf row 0
    nc.sync.dma_start(v1[:, 0:1, :], v0[:, h0:h0 + 1, :])
    # image row h0: primary = half1 buf row 1 ; -> half0 buf row h0+1
    nc.sync.dma_start(v0[:, h0 + 1:h0 + 2, :], v1[:, 1:2, :])


# ------------------------------------------------------------------ driver --

def kernel(x, params):
    x = np.asarray(x, np.float32)
    for (w, b, gmm, bt) in params:
        assert np.all(np.asarray(gmm) == 1.0) and np.all(np.asarray(bt) == 0.0), \
            "kernel assumes gamma=1, beta=0"
    H0 = x.shape[2]
    if 'nc' not in _CACHE:
        _CACHE['nc'] = build_program(H0)
    nc = _CACHE['nc']
    ws = prep_weights(params, H0)
    in_maps = []
    for i in range(N_CORES):
        m = {'im2col0': prep_im2col(x[i], H0)}
        m.update(ws)
        in_maps.append(m)
    res = run_bass_kernel_spmd(nc, in_maps, core_ids=list(range(N_CORES)))
    _CACHE['last_result'] = res
    Hf = make_geom(H0)[10]['Hout']
    out = np.stack([res.results[i]['out'].reshape(11, Hf, Hf)
                    for i in range(N_CORES)])
    return out.astype(np.float32)


if __name__ == "__main__":
    import pickle, time
    x = np.load('/root/problem/x.npy')
    params = pickle.load(open('/root/problem/params.pkl', 'rb'))
    ref = np.load('/root/problem/ref_out.npy')
    t0 = time.time()
    out = kernel(x, params)
    print("kernel() wall", time.time() - t0)
    err = np.abs(out - ref)
    print("abs max err", err.max(),
          "rel l2", np.linalg.norm(out - ref) / np.linalg.norm(ref),
          "bad pixels", (err > 1e-3).sum())


# revision 24
# speedup vs baseline: 1.0695x; 1.0132x over previous
"""Trainium2 Bass kernel for an 11-layer binarized encoder-decoder CNN.

Contract: kernel(**inputs) takes the FULL inputs from setup_inputs()
(x: (8,3,256,256) f32, params: tuple of 11 (w, b, gamma, beta)) and
returns the FULL output (8, 11, 253, 253) f32.

Strategy (pure data parallel, 8 NeuronCores, one image per core):
- All conv layers run on the tensor engine as 9-tap matmul accumulations
  into PSUM (conv-transpose as 4 output-parity classes with 1/2/4 taps).
- Binarized weights / sign activations are exactly representable in bf16,
  and every intermediate conv output is an exact small integer, so all
  binary-layer arithmetic on the PE is EXACT in bf16 with fp32 PSUM.
- Training-mode BN + binary_tanh folds to `sign(y - mean_c)` because
  gamma=1, beta=0 (asserted); conv bias cancels inside BN entirely.
- Channel means need the batch-global sum: each layer does a tiny
  [128,2] AllReduce across the 8 cores (sync-BN), then a vector-engine
  pass computes (raw >= t)*2-1 into the next layer's padded input plane.
- K=64 layers split the image into two halves mapped to PE row-groups
  (0,0)/(64,0) so two tap-streams run concurrently on the 128x128 array;
  layer 0 (K=27) uses a host-built im2col and 4 concurrent row-groups.
"""

import numpy as np
from contextlib import ExitStack

import ml_dtypes
import concourse.bass as bass
import concourse.bacc as bacc
import concourse.mybir as mybir
import concourse.tile as tile
from concourse.bass_utils import run_bass_kernel_spmd

F32 = mybir.dt.float32
BF16 = mybir.dt.bfloat16
AF = mybir.ActivationFunctionType
ALU = mybir.AluOpType

N_CORES = 8

# ---------------------------------------------------------------- geometry --
# Layer table: (kind, stride, act, Cin, Cout, Hin) ; Hout derived.
# plane layout kinds: 'halves' (C=64 on partitions 0-63 / 64-127 by image
# half), 'full' (C=128), 'chunks' (C=256 as two 128-channel planes).

def make_geom(H0=256):
    g = []
    # returns list of layer dicts L0..L10
    def conv_out(H, s):
        return (H + 2 - 3) // s + 1
    sizes = [H0]
    cfg = [('conv', 1, True, 3, 64), ('bin', 1, True, 64, 64), ('bin', 2, True, 64, 128),
           ('bin', 1, True, 128, 128), ('bin', 2, True, 128, 256), ('bin', 1, True, 256, 256),
           ('binT', 2, True, 256, 128), ('bin', 1, True, 128, 128), ('binT', 2, True, 128, 64),
           ('bin', 1, True, 64, 64), ('bin', 1, False, 64, 11)]
    H = H0
    for (kind, s, act, ci, co) in cfg:
        Hin = H
        H = conv_out(H, s) if kind != 'binT' else 2 * H - 1
        g.append(dict(kind=kind, s=s, act=act, Cin=ci, Cout=co, Hin=Hin, Hout=H))
    return g


def plane_layout(C, H):
    """How an H x H C-channel +/-1 plane is stored in the sbuf arena."""
    if C == 64:
        h0 = (H + 1) // 2          # rows in half 0
        rows = h0 + 2              # stored rows per half (1 halo/overlap each side)
        return dict(kind='halves', C=C, H=H, W=H, pitch=H + 2, rows=rows,
                    h0=h0, elems=rows * (H + 2))
    if C == 128:
        return dict(kind='full', C=C, H=H, W=H, pitch=H + 2, rows=H + 2,
                    elems=(H + 2) * (H + 2))
    if C == 256:
        return dict(kind='chunks', C=C, H=H, W=H, pitch=H + 2, rows=H + 2,
                    elems=2 * (H + 2) * (H + 2))
    raise ValueError(C)


# conv-transpose parity-class tap tables: class (a,b) -> list of (ky,kx,di,dj)
def convt_classes():
    out = {}
    for a in (0, 1):
        for b in (0, 1):
            kys = [(1, 0)] if a == 0 else [(0, 0), (2, 1)]
            kxs = [(1, 0)] if b == 0 else [(0, 0), (2, 1)]
            out[(a, b)] = [(ky, kx, di, dj) for (ky, di) in kys for (kx, dj) in kxs]
    return out

CT_CLASSES = convt_classes()


# ------------------------------------------------------------ host weights --

def prep_weights(params, H0=256):
    geom = make_geom(H0)
    ws = {}
    for i, ((kind, *_), p) in enumerate(zip([(g['kind'],) for g in geom], params)):
        pass
    # L0: [27,64] f32 replicated at partition bases 0/32/64/96 -> [128,64]
    w0 = np.asarray(params[0][0], np.float32)           # (64,3,3,3) = (O,I,ky,kx)
    a0 = w0.transpose(2, 3, 1, 0).reshape(27, 64)       # row = (dy*3+dx)*3+cin
    w0sb = np.zeros((128, 64), np.float32)
    for q in range(4):
        w0sb[32 * q:32 * q + 27] = a0
    ws['w0'] = w0sb
    for i in range(1, 11):
        g = geom[i]
        w = np.asarray(params[i][0], np.float32)
        wq = np.where(w >= 0, 1.0, -1.0).astype(np.float32)
        if g['kind'] == 'binT':
            # torch convT weight (I,O,3,3); effective kernel wt[o,i,ky,kx] = wq[i,o,2-ky,2-kx]
            wt = wq.transpose(1, 0, 2, 3)[:, :, ::-1, ::-1]
        else:
            wt = wq                                      # (O,I,ky,kx)
        O, I = wt.shape[0], wt.shape[1]
        arr = wt.transpose(1, 2, 3, 0).reshape(I, 9 * O)  # [cin, tap*O + o]
        if I == 64:
            sb = np.concatenate([arr, arr], axis=0)       # duplicate for row-groups
        elif I == 128:
            sb = arr
        elif I == 256:
            sb = np.concatenate([arr[:128], arr[128:]], axis=1)  # [128, 2*9*O]
        else:
            raise ValueError(I)
        ws[f'w{i}'] = np.ascontiguousarray(sb.astype(ml_dtypes.bfloat16))
    return ws


def prep_im2col(x_img, H0=256):
    """x_img (3,H,H) f32 -> [128, (H/4)*H] f32 im2col, 4 quarter row-groups."""
    H = H0
    xp = np.pad(x_img, ((0, 0), (1, 1), (1, 1)))
    win = np.lib.stride_tricks.sliding_window_view(xp, (3, 3), axis=(1, 2))
    arr = win.transpose(3, 4, 0, 1, 2).reshape(27, H, H)   # row=(dy*3+dx)*3+cin
    q = H // 4
    out = np.zeros((128, q * H), np.float32)
    for k in range(4):
        out[32 * k:32 * k + 27] = arr[:, k * q:(k + 1) * q, :].reshape(27, q * H)
    return out


# ------------------------------------------------------------- the program --

_CACHE = {}

def build_program(H0=256):
    geom = make_geom(H0)
    Hf = geom[10]['Hout']                      # final H (253 for H0=256)
    NPIXF = Hf * Hf

    nc = bacc.Bacc("TRN2", target_bir_lowering=False, debug=False,
                   num_devices=N_CORES)

    # ---- dram I/O
    im2col_d = nc.dram_tensor("im2col0", [128, (H0 // 4) * H0], F32, kind="ExternalInput")
    wd = {0: nc.dram_tensor("w0", [128, 64], F32, kind="ExternalInput")}
    for i in range(1, 11):
        g = geom[i]
        I, O = g['Cin'], g['Cout']
        cols = 9 * O * (2 if I == 256 else 1)
        wd[i] = nc.dram_tensor(f"w{i}", [128, cols], BF16, kind="ExternalInput")
    out_d = nc.dram_tensor("out", [11, NPIXF], F32, kind="ExternalOutput")
    dbg_d = nc.dram_tensor("dbg", [128, 24], F32, kind="ExternalOutput")
    raw0_d = nc.dram_tensor("raw0", [128, (H0 * H0) // 2], F32)   # L0 raw, halves

    AR_ELEMS = max(pl['elems'] for pl in
                   [plane_layout(geom[i]['Cin'], geom[i]['Hin']) for i in range(1, 11)])
    RAW_BYTES = 65536 if H0 == 256 else max(8192, (H0 * H0) // 2 * 2)

    with tile.TileContext(nc) as tc, ExitStack() as ctx:
        wpool = ctx.enter_context(tc.tile_pool(name="w", bufs=1))
        arena_p = ctx.enter_context(tc.tile_pool(name="arena", bufs=1))
        rawp = ctx.enter_context(tc.tile_pool(name="raw", bufs=1))
        stage = ctx.enter_context(tc.tile_pool(name="stage", bufs=2))
        psum = ctx.enter_context(tc.tile_pool(name="psum", bufs=6, space="PSUM"))
        psumd = ctx.enter_context(tc.tile_pool(name="psumd", bufs=1, space="PSUM"))
        small = ctx.enter_context(tc.tile_pool(name="small", bufs=4))
        dram = ctx.enter_context(tc.tile_pool(name="dram", bufs=4, space="DRAM"))

        # ---- persistent tiles (im2col + w0 stream first so L0 starts early)
        arena = arena_p.tile([128, AR_ELEMS], BF16)
        H0q = (H0 // 4) * H0
        im2col_sb0 = rawp.tile([128, H0q], F32, tag="raw", name="im2col_sb0")
        wsb = {0: wpool.tile([128, 64], F32, tag="w0", name="w0sb")}
        nc.sync.dma_start(wsb[0][:], wd[0][:])
        for ch in range(8):
            c0 = (H0q // 8) * ch
            c1 = (H0q // 8) * (ch + 1)
            nc.sync.dma_start(im2col_sb0[:, c0:c1], im2col_d[:, c0:c1])
        for i in range(1, 11):
            shp = wd[i].shape
            wsb[i] = wpool.tile(list(shp), BF16, tag=f"w{i}", name=f"w{i}sb")
            nc.sync.dma_start(wsb[i][:], wd[i][:])
        dbg = wpool.tile([128, 24], F32, tag="dbg")
        # two persistent ping-pong PSUM tiles for PE warm-keeper matmuls:
        # same-engine WAW needs no semaphores, alternating banks avoids
        # same-bank drain/fill overlap.
        dumA = psumd.tile([128, 512], F32, tag="dumA", name="dumA")
        dumB = psumd.tile([128, 512], F32, tag="dumB", name="dumB")

        def warm_pe(n, anchor):
            """Keep the PE busy through a stats-exchange gap. `anchor` is a
            bf16 [P, >=512] AP written by this layer's evacs: the RAW dep
            pins the dummies to this gap (else the scheduler hoists them)."""
            P = anchor.shape[0]
            N = min(512, anchor.shape[1])
            for _k in range(n):
                dp = dumA if _k % 2 == 0 else dumB
                nc.tensor.matmul(dp[0:128, 0:N], wsb[1][0:P, 0:128],
                                 anchor[0:P, 0:N],
                                 start=True, stop=True)
        nc.vector.memset(dbg[:], 0.0)

        # ---- dummy collective to warm up the CC path (overlaps input DMAs)
        warm_in = dram.tile([128, 2], F32)
        warm_out = dram.tile([128, 2], F32)
        warm_sb = small.tile([128, 2], F32)
        nc.vector.memset(warm_sb[:], 0.0)
        nc.sync.dma_start(warm_in[:], warm_sb[:])
        nc.gpsimd.collective_compute(
            "AllReduce", ALU.add, replica_groups=[list(range(N_CORES))],
            ins=[warm_in[:].opt()], outs=[warm_out[:].opt()])

        # =================================================================
        # helpers
        # =================================================================
        def plane_view(pl, h_or_c):
            """3-D [Cpart, rows, pitch] AP into the arena for half/chunk."""
            if pl['kind'] == 'halves':
                p0 = 64 * h_or_c
                v = arena[p0:p0 + 64, 0:pl['rows'] * pl['pitch']]
                return v.rearrange("p (r c) -> p r c", c=pl['pitch'])
            if pl['kind'] == 'full':
                v = arena[:, 0:pl['rows'] * pl['pitch']]
                return v.rearrange("p (r c) -> p r c", c=pl['pitch'])
            # chunks
            off = h_or_c * pl['rows'] * pl['pitch']
            v = arena[:, off:off + pl['rows'] * pl['pitch']]
            return v.rearrange("p (r c) -> p r c", c=pl['pitch'])

        def halo_memset(pl):
            """Zero the 1-px halo ring of a plane (and for halves the outer
            halo rows); interior is written by the sign pass."""
            if pl['kind'] in ('full', 'chunks'):
                n = 2 if pl['kind'] == 'chunks' else 1
                for c in range(n):
                    v = plane_view(pl, c)
                    nc.gpsimd.memset(v[:, 0:1, :], 0.0)
                    nc.gpsimd.memset(v[:, pl['rows'] - 1:pl['rows'], :], 0.0)
                    nc.gpsimd.memset(v[:, :, 0:1], 0.0)
                    nc.gpsimd.memset(v[:, :, pl['pitch'] - 1:pl['pitch']], 0.0)
            else:
                v0, v1 = plane_view(pl, 0), plane_view(pl, 1)
                nc.gpsimd.memset(v0[:, 0:1, :], 0.0)                    # top halo
                bh = pl['H'] - pl['h0'] + 1
                nc.gpsimd.memset(v1[:, bh:bh + 1, :], 0.0)  # bottom halo
                for v in (v0, v1):
                    nc.gpsimd.memset(v[:, :, 0:1], 0.0)
                    nc.gpsimd.memset(v[:, :, pl['pitch'] - 1:pl['pitch']], 0.0)

        def cc_allreduce(payload_sb):
            """AllReduce a [128,2] f32 sbuf tile across the 8 cores; returns
            a [128,2] sbuf tile with the global sums."""
            cin = dram.tile([128, 2], F32)
            cout = dram.tile([128, 2], F32)
            nc.sync.dma_start(cin[:], payload_sb[:])
            nc.gpsimd.collective_compute(
                "AllReduce", ALU.add, replica_groups=[list(range(N_CORES))],
                ins=[cin[:].opt()], outs=[cout[:].opt()])
            g = small.tile([128, 2], F32, tag="gsum")
            nc.sync.dma_start(g[:], cout[:])
            return g

        # =================================================================
        # Layer 0: im2col conv (K=27, 4 row-group quarters), fp32
        # =================================================================
        g0 = geom[0]
        H = g0['Hout']; W = H
        q = H // 4                       # rows per quarter
        im2col_sb = im2col_sb0

        rows_per_t = max(1, 512 // W)
        acc0 = small.tile([64, 256], F32, tag="acc")
        n_evac = 0
        hp = (H * H) // 2                # pixels per half
        for r0 in range(0, q, rows_per_t):
            nr = min(rows_per_t, q - r0)
            pt = [psum.tile([64, nr * W], F32, tag="ps", name=f"p0_{_k}") for _k in range(4)]
            for k in range(4):
                nc.tensor.matmul(
                    pt[k][:], wsb[0][32 * k:32 * k + 27, 0:64],
                    im2col_sb[32 * k:32 * k + 27, r0 * W:(r0 + nr) * W],
                    start=True, stop=True,
                    tile_position=(32 * k, 0))
            for k in range(4):
                st = stage.tile([64, rows_per_t * W], F32, tag="stage")
                nc.scalar.activation(st[0:64, 0:nr * W], pt[k][:], AF.Copy,
                                     accum_out=acc0[:, n_evac:n_evac + 1])
                n_evac += 1
                half = k // 2
                off = (k % 2) * q * W + r0 * W
                nc.sync.dma_start(
                    raw0_d[64 * half:64 * half + 64, off:off + nr * W],
                    st[0:64, 0:nr * W])

        warm_pe(200, acc0[:, 0:n_evac].bitcast(BF16)[:, 1:2 * n_evac:2])
        # ---- stats + threshold for L0
        loc = small.tile([128, 2], F32, tag="pay")
        nc.vector.memset(loc[:], 0.0)
        nc.vector.tensor_reduce(loc[0:64, 0:1], acc0[:, 0:n_evac],
                                mybir.AxisListType.X, ALU.add)
        gs = cc_allreduce(loc)
        t0v = small.tile([128, 1], F32, tag="thr")
        nc.vector.tensor_scalar(t0v[0:64, :], gs[0:64, 0:1],
                                1.0 / (N_CORES * H * W), None, ALU.mult)
        nc.sync.dma_start(t0v[64:128, :], t0v[0:64, :])   # dup for half1 partitions
        nc.vector.tensor_copy(dbg[:, 0:1], t0v[:])

        # ---- sign pass L0: raw0 (dram, f32, halves layout) -> plane P1
        pl1 = plane_layout(64, H)
        CH = max(W, (2048 // W) * W)
        for off in range(0, hp, CH):
            n = min(CH, hp - off)
            st = stage.tile([128, 2048], F32, tag="stage")
            nc.sync.dma_start(st[:, 0:n], raw0_d[:, off:off + n])
            s2 = stage.tile([128, 4096], BF16, tag="stage2")
            nc.vector.tensor_scalar(s2[:, 0:n], st[:, 0:n], t0v[:], 2.0,
                                    ALU.is_ge, ALU.mult)
            r0 = off // W; nr = n // W
            for h in (0, 1):
                v = plane_view(pl1, h)
                nc.vector.tensor_scalar(
                    v[:, 1 + r0:1 + r0 + nr, 1:1 + W],
                    s2[64 * h:64 * h + 64, 0:n].rearrange("p (r c) -> p r c", c=W),
                    1.0, None, ALU.subtract)
        halo_memset(pl1)
        # overlap rows between halves (image rows h0-1 and h0)
        fix_overlaps(nc, plane_view, pl1)

        # =================================================================
        # Layers 1..9
        # =================================================================
        for li in range(1, 10):
            g = geom[li]
            pin = plane_layout(g['Cin'], g['Hin'])
            Ho, Wo, Co = g['Hout'], g['Hout'], g['Cout']
            npix = Ho * Wo

            acc = small.tile([128, 256], F32, tag="acc")
            n_evac = 0
            raw_cols = (npix // (2 if Co == 64 else 1)) * (2 if Co == 256 else 1)
            rawt = rawp.tile([128, RAW_BYTES // 2], BF16, tag="raw")

            def evac(pt, dst_ap, cpart):
                nonlocal n_evac
                if isinstance(pt, tuple):
                    nc.scalar.activation(dst_ap, pt[0], AF.Copy)
                    nc.vector.scalar_tensor_tensor(
                        out=dst_ap, in0=pt[1], scalar=1.0, in1=dst_ap,
                        op0=ALU.mult, op1=ALU.add,
                        accum_out=acc[cpart, n_evac:n_evac + 1])
                else:
                    nc.scalar.activation(dst_ap, pt, AF.Copy,
                                         accum_out=acc[cpart, n_evac:n_evac + 1])
                n_evac += 1

            if g['kind'] == 'bin':
                conv_bin(nc, psum, wsb[li], g, pin, plane_view, rawt, evac)
            else:
                conv_t(nc, psum, wsb[li], g, pin, plane_view, rawt, evac)

            warm_pe(60 + npix // 800, rawt[:, 0:min(512, ((Ho - (Ho + 1) // 2) * Wo) if Co == 64 else npix)])
            # ---- stats + threshold (halo memsets first: overlap cc latency)
            halo_memset(plane_layout(Co, Ho))
            loc = small.tile([128, 2], F32, tag="pay")
            nc.vector.memset(loc[:], 0.0)
            if Co == 256:
                nc.vector.tensor_reduce(loc[:, 0:1], acc[:, 0:n_evac:2],
                                        mybir.AxisListType.X, ALU.add)
                nc.vector.tensor_reduce(loc[:, 1:2], acc[:, 1:n_evac:2],
                                        mybir.AxisListType.X, ALU.add)
            else:
                nc.vector.tensor_reduce(loc[0:Co, 0:1], acc[0:Co, 0:n_evac],
                                        mybir.AxisListType.X, ALU.add)
            gs = cc_allreduce(loc)
            tv = small.tile([128, 2], F32, tag="thr")
            inv = 1.0 / (N_CORES * npix)
            if Co == 64:
                nc.vector.tensor_scalar(tv[0:64, 0:1], gs[0:64, 0:1], inv, None, ALU.mult)
                nc.sync.dma_start(tv[64:128, 0:1], tv[0:64, 0:1])
            elif Co == 128:
                nc.vector.tensor_scalar(tv[:, 0:1], gs[:, 0:1], inv, None, ALU.mult)
            else:
                nc.vector.tensor_scalar(tv[:, 0:2], gs[:, 0:2], inv, None, ALU.mult)
            nc.vector.tensor_copy(dbg[:, 2 * li:2 * li + 1], tv[:, 0:1])
            nc.vector.tensor_copy(dbg[:, 2 * li + 1:2 * li + 2], gs[:, 0:1])

            # ---- sign pass -> next plane
            pout = plane_layout(Co, Ho)
            sign_pass(nc, stage, rawt, tv, pout, plane_view, Ho, Wo, Co)
            if pout['kind'] == 'halves':
                fix_overlaps(nc, plane_view, pout)

        # =================================================================
        # Layer 10: conv + full BN (no activation) -> output
        # =================================================================
        g = geom[10]
        pin = plane_layout(64, g['Hin'])
        Ho = g['Hout']; Wo = Ho; npix = Ho * Wo
        h0 = (Ho + 1) // 2
        # raw10: quarters of the image on partition bases 0/32/64/96, f32.
        # Quarter boundaries are aligned to evac tiles (split each half's
        # tile list in two) so a tile never straddles a quarter.
        rows_per_t = max(1, 512 // Wo)
        h0in = pin['h0']
        tl = {h: [(y0h + yy, min(rows_per_t, nrh - yy))
                  for yy in range(0, nrh, rows_per_t)]
              for h, (y0h, nrh) in enumerate([(0, h0), (h0, Ho - h0)])}
        qrows = []
        tile_q = {}
        for h in (0, 1):
            n1 = (len(tl[h]) + 1) // 2
            for gi, seg in enumerate([tl[h][:n1], tl[h][n1:]]):
                qi = 2 * h + gi
                qrows.append((seg[0][0], sum(nr for _, nr in seg)) if seg else (0, 0))
                for t in seg:
                    tile_q[t] = qi
        raw10 = rawp.tile([128, RAW_BYTES // 4], F32, tag="raw")
        nc.gpsimd.memset(raw10[:], 0.0)
        acc = small.tile([11, 256], F32, tag="acc")
        accq = small.tile([11, 256], F32, tag="accq")
        n_evac = 0

        nt10 = max(len(tl[0]), len(tl[1]))
        for ti in range(nt10):
            pts = {}
            for h in (0, 1):
                if ti < len(tl[h]):
                    pts[h] = (psum.tile([11, tl[h][ti][1] * Wo], F32, tag="ps",
                                        name=f"pt10_{h}"),) + tl[h][ti]
            for t9 in range(9):
                dy, dx = t9 // 3, t9 % 3
                for h, (pt, y, nr) in pts.items():
                    vin = plane_view(pin, h)
                    buf0 = y + dy if h == 0 else (y + dy - 1) - (h0in - 1)
                    rhs = vin[:, buf0:buf0 + nr, dx:dx + Wo]
                    nc.tensor.matmul(pt[:], wsb[10][64 * h:64 * h + 64,
                                                    t9 * 11:t9 * 11 + 11],
                                     rhs, start=(t9 == 0), stop=(t9 == 8),
                                     tile_position=(64 * h, 0))
            for h, (pt, y, nr) in pts.items():
                qi = tile_q[(y, nr)]
                off = (y - qrows[qi][0]) * Wo
                nc.scalar.activation(
                    raw10[32 * qi:32 * qi + 11, off:off + nr * Wo], pt[:], AF.Copy,
                    accum_out=acc[:, n_evac:n_evac + 1])
                sq = stage.tile([11, 512], F32, tag="sq")
                rsl = raw10[32 * qi:32 * qi + 11, off:off + nr * Wo]
                nc.vector.scalar_tensor_tensor(
                    out=sq[:, 0:nr * Wo], in0=rsl, scalar=1.0, in1=rsl,
                    op0=ALU.mult, op1=ALU.mult,
                    accum_out=accq[:, n_evac:n_evac + 1])
                n_evac += 1

        warm_pe(30, raw10[:].bitcast(BF16)[:, 1:1024:2])
        loc = small.tile([128, 2], F32, tag="pay")
        nc.vector.memset(loc[:], 0.0)
        nc.vector.tensor_reduce(loc[0:11, 0:1], acc[:, 0:n_evac],
                                mybir.AxisListType.X, ALU.add)
        nc.vector.tensor_reduce(loc[0:11, 1:2], accq[:, 0:n_evac],
                                mybir.AxisListType.X, ALU.add)
        gs = cc_allreduce(loc)
        inv = 1.0 / (N_CORES * npix)
        m = small.tile([128, 1], F32, tag="m")
        qm = small.tile([128, 1], F32, tag="qm")
        nc.vector.tensor_scalar(m[0:11, :], gs[0:11, 0:1], inv, None, ALU.mult)
        nc.vector.tensor_scalar(qm[0:11, :], gs[0:11, 1:2], inv, None, ALU.mult)
        var = small.tile([128, 1], F32, tag="var")
        nc.vector.tensor_tensor(var[0:11, :], m[0:11, :], m[0:11, :], ALU.mult)
        nc.vector.tensor_tensor(var[0:11, :], qm[0:11, :], var[0:11, :], ALU.subtract)
        nc.vector.tensor_scalar(var[0:11, :], var[0:11, :], 1e-4, None, ALU.add)
        sd = small.tile([128, 1], F32, tag="sd")
        nc.scalar.activation(sd[0:11, :], var[0:11, :], AF.Sqrt)
        rs = small.tile([128, 4], F32, tag="rs")
        nc.vector.memset(rs[:], 0.0)
        nc.vector.reciprocal(rs[0:11, 0:1], sd[0:11, :])
        # bias = -m*rs ; out = raw*rs + bias
        nc.vector.tensor_tensor(rs[0:11, 1:2], m[0:11, :], rs[0:11, 0:1], ALU.mult)
        nc.vector.tensor_scalar(rs[0:11, 1:2], rs[0:11, 1:2], -1.0, None, ALU.mult)
        nc.vector.tensor_copy(dbg[0:11, 20:21], m[0:11, :])
        nc.vector.tensor_copy(dbg[0:11, 21:22], rs[0:11, 0:1])
        for qi in (1, 2, 3):
            nc.sync.dma_start(rs[32 * qi:32 * qi + 11, 0:2], rs[0:11, 0:2])
        # affine: one chunked op across all 4 partition groups at once
        # (garbage rows have scale=bias=0 and raw10 was memset -> output 0)
        maxcols = max(nrq for _, nrq in qrows) * Wo
        for off in range(0, maxcols, 2048):
            n = min(2048, maxcols - off)
            ot = stage.tile([128, 2048], F32, tag="stage")
            nc.scalar.activation(ot[0:107, 0:n], raw10[0:107, off:off + n],
                                 AF.Identity, bias=rs[0:107, 1:2],
                                 scale=rs[0:107, 0:1])
            for qi in range(4):
                ncols = qrows[qi][1] * Wo
                lo, hi = off, min(off + n, ncols)
                if lo >= hi:
                    continue
                nc.sync.dma_start(
                    out_d[0:11, qrows[qi][0] * Wo + lo: qrows[qi][0] * Wo + hi],
                    ot[32 * qi:32 * qi + 11, lo - off:hi - off])
        nc.sync.dma_start(dbg_d[:], dbg[:])

    nc.compile()
    return nc


# ---------------------------------------------------------- conv emitters --

def conv_bin(nc, psum, wsb, g, pin, plane_view, rawt, evac):
    """Standard 3x3 conv (stride 1 or 2). Emits matmuls + evacs."""
    s, Ci, Co, Ho = g['s'], g['Cin'], g['Cout'], g['Hout']
    Wo = Ho
    rows_per_t = max(1, 512 // Wo)
    if pin['kind'] == 'halves':
        # output half h comes from input half h; interleave for row-groups
        h0o = (Ho + 1) // 2                       # out rows in half 0
        halves = [(0, h0o), (h0o, Ho - h0o)]
        nt = max(len(range(0, hh[1], rows_per_t)) for hh in halves)
        for ti in range(nt):
            pts = {}
            for h, (y0h, nrh) in enumerate(halves):
                yy = ti * rows_per_t
                if yy >= nrh:
                    continue
                nr = min(rows_per_t, nrh - yy)
                pts[h] = (psum.tile([Co, nr * Wo], F32, tag="ps", name=f"pb{h}"), yy, nr)
            for t9 in range(9):
                dy, dx = t9 // 3, t9 % 3
                for h, (pt, yy, nr) in pts.items():
                    y = halves[h][0] + yy
                    vin = plane_view(pin, h)
                    # input buf row of out row y, tap dy: s*y+dy-1 - buf0
                    buf0_img = -1 if h == 0 else s * halves[1][0] - 1
                    br = s * y + dy - 1 - buf0_img
                    rhs = vin[:, br:br + (nr - 1) * s + 1:s,
                              dx:dx + (Wo - 1) * s + 1:s]
                    nc.tensor.matmul(pt[:], wsb[64 * h:64 * h + 64,
                                                t9 * Co:t9 * Co + Co],
                                     rhs, start=(t9 == 0), stop=(t9 == 8),
                                     tile_position=(64 * h, 0))
            for h, (pt, yy, nr) in pts.items():
                y = halves[h][0] + yy
                dst = raw_dst(rawt, Co, Ho, Wo, y, nr)
                evac(pt[:], dst, slice(0, Co))
    else:
        # full or chunks input: split K into 64-row halves on alternating PE
        # row-groups (LDWEIGHTS of one half overlaps the other's matmul);
        # each row-group accumulates into its own PSUM bank, summed at evac.
        kc = 2 if Ci == 256 else 1
        mc = 2 if Co == 256 else 1
        for y in range(0, Ho, rows_per_t):
            nr = min(rows_per_t, Ho - y)
            for mi in range(mc):
                Mo = min(128, Co)
                pA = psum.tile([Mo, nr * Wo], F32, tag="ps", name="pA")
                pB = psum.tile([Mo, nr * Wo], F32, tag="ps", name="pB")
                nmm = [0, 0]
                ntot = 9 * kc
                for ki in range(kc):
                    vin = plane_view(pin, ki if pin['kind'] == 'chunks' else 0)
                    for t9 in range(9):
                        dy, dx = t9 // 3, t9 % 3
                        br = s * y + dy
                        col0 = (ki * 9 + t9) * Co + mi * 128 if Ci == 256 else t9 * Co + mi * 128
                        for u in (0, 1):
                            rg = (2 * ki + t9 + u) % 2 if kc == 2 else (t9 + u) % 2
                            rg = u
                            pt = (pA, pB)[u]
                            rhs = vin[64 * u:64 * u + 64, br:br + (nr - 1) * s + 1:s,
                                      dx:dx + (Wo - 1) * s + 1:s]
                            nc.tensor.matmul(pt[:], wsb[64 * u:64 * u + 64, col0:col0 + Mo],
                                             rhs, start=(nmm[u] == 0),
                                             stop=(nmm[u] == ntot - 1),
                                             tile_position=(64 * u, 0))
                            nmm[u] += 1
                dst = raw_dst(rawt, Co, Ho, Wo, y, nr, mi)
                evac((pA, pB), dst, slice(0, 128) if Co >= 128 else slice(0, Co))


def conv_t(nc, psum, wsb, g, pin, plane_view, rawt, evac):
    """Conv-transpose stride 2 via 4 output parity classes."""
    Ci, Co, Hi, Ho = g['Cin'], g['Cout'], g['Hin'], g['Hout']
    Wo = Ho
    kc = 2 if Ci == 256 else 1
    for (a, b), taps in CT_CLASSES.items():
        ia = (Ho - a + 1) // 2          # class rows
        jb = (Ho - b + 1) // 2          # class cols
        rows_per_t = max(1, 512 // jb)
        for i0 in range(0, ia, rows_per_t):
            nr = min(rows_per_t, ia - i0)
            pA = psum.tile([Co, nr * jb], F32, tag="ps", name="pA")
            pB = psum.tile([Co, nr * jb], F32, tag="ps", name="pB")
            pt2 = (pA, pB)
            nmm = [0, 0]
            ntot = len(taps) * kc
            for ki in range(kc):
                vin = plane_view(pin, ki if pin['kind'] == 'chunks' else 0)
                for (ky, kx, di, dj) in taps:
                    col0 = (ki * 9 + (ky * 3 + kx)) * Co if Ci == 256 else (ky * 3 + kx) * Co
                    for u in (0, 1):
                        rhs = vin[64 * u:64 * u + 64, 1 + i0 + di:1 + i0 + di + nr,
                                  1 + dj:1 + dj + jb]
                        nc.tensor.matmul(pt2[u][:], wsb[64 * u:64 * u + 64, col0:col0 + Co],
                                         rhs, start=(nmm[u] == 0),
                                         stop=(nmm[u] == ntot - 1),
                                         tile_position=(64 * u, 0))
                        nmm[u] += 1
            # evac into raw, strided by parity class
            if Co == 64:
                # halves raw layout; class rows may straddle half boundary
                h0 = (Ho + 1) // 2
                rows = [2 * (i0 + k) + a for k in range(nr)]
                segs = []
                k = 0
                while k < nr:
                    h = 0 if rows[k] < h0 else 1
                    k2 = k
                    while k2 < nr and (0 if rows[k2] < h0 else 1) == h:
                        k2 += 1
                    segs.append((k, k2, h))
                    k = k2
                for (k, k2, h) in segs:
                    y0 = rows[k] - (0 if h == 0 else h0)
                    v = rawt[64 * h:64 * h + 64, 0:((h0 if h == 0 else Ho - h0) * Wo)]
                    v3 = v.rearrange("p (r c) -> p r c", c=Wo)
                    dst = v3[:, y0:y0 + 2 * (k2 - k - 1) + 1:2, b:b + 2 * (jb - 1) + 1:2]
                    evac((pA[0:64, k * jb:k2 * jb].rearrange("p (r c) -> p r c", c=jb),
                          pB[0:64, k * jb:k2 * jb].rearrange("p (r c) -> p r c", c=jb)),
                         dst, slice(0, 64))
            else:
                v3 = rawt[:, 0:Ho * Wo].rearrange("p (r c) -> p r c", c=Wo)
                dst = v3[:, a + 2 * i0:a + 2 * (i0 + nr - 1) + 1:2, b:b + 2 * (jb - 1) + 1:2]
                evac((pA[:].rearrange("p (r c) -> p r c", c=jb),
                      pB[:].rearrange("p (r c) -> p r c", c=jb)), dst, slice(0, Co))


def raw_dst(rawt, Co, Ho, Wo, y, nr, mi=0):
    """AP into the raw tile for output rows y..y+nr (contiguous layout)."""
    if Co == 64:
        h0 = (Ho + 1) // 2
        h = 0 if y < h0 else 1
        y0 = y - (0 if h == 0 else h0)
        return rawt[64 * h:64 * h + 64, y0 * Wo:(y0 + nr) * Wo]
    if Co == 128:
        return rawt[:, y * Wo:(y + nr) * Wo]
    # Co == 256: chunk mi at offset mi*npix
    npix = Ho * Wo
    return rawt[:, mi * npix + y * Wo: mi * npix + (y + nr) * Wo]


def sign_pass(nc, stage, rawt, tv, pout, plane_view, Ho, Wo, Co):
    """(raw >= t)*2-1 -> padded plane interior (bf16)."""
    if pout['kind'] == 'halves':
        h0 = (Ho + 1) // 2
        for h, (r0, nrh) in enumerate([(0, h0), (h0, Ho - h0)]):
            base = 64 * h
            CH = max(Wo, (4096 // Wo) * Wo)
            for off in range(0, nrh * Wo, CH):
                n = min(CH, nrh * Wo - off)
                s2 = stage.tile([128, 4096], BF16, tag="stage2")
                nc.vector.tensor_scalar(s2[base:base + 64, 0:n],
                                        rawt[base:base + 64, off:off + n],
                                        tv[base:base + 64, 0:1], 2.0,
                                        ALU.is_ge, ALU.mult)
                v = plane_view(pout, h)
                rr = off // Wo
                nc.vector.tensor_scalar(
                    v[:, 1 + rr:1 + rr + n // Wo, 1:1 + Wo],
                    s2[base:base + 64, 0:n].rearrange("p (r c) -> p r c", c=Wo),
                    1.0, None, ALU.subtract)
    elif pout['kind'] == 'full':
        CH = max(Wo, (4096 // Wo) * Wo)
        for off in range(0, Ho * Wo, CH):
            n = min(CH, Ho * Wo - off)
            s2 = stage.tile([128, 4096], BF16, tag="stage2")
            nc.vector.tensor_scalar(s2[:, 0:n], rawt[:, off:off + n],
                                    tv[:, 0:1], 2.0, ALU.is_ge, ALU.mult)
            v = plane_view(pout, 0)
            rr = off // Wo
            nc.vector.tensor_scalar(
                v[:, 1 + rr:1 + rr + n // Wo, 1:1 + Wo],
                s2[:, 0:n].rearrange("p (r c) -> p r c", c=Wo),
                1.0, None, ALU.subtract)
    else:   # chunks (Co=256)
        npix = Ho * Wo
        CH = max(Wo, (4096 // Wo) * Wo)
        for c in range(2):
            for off in range(0, npix, CH):
                n = min(CH, npix - off)
                s2 = stage.tile([128, 4096], BF16, tag="stage2")
                nc.vector.tensor_scalar(s2[:, 0:n], rawt[:, c * npix + off:c * npix + off + n],
                                        tv[:, c:c + 1], 2.0, ALU.is_ge, ALU.mult)
                v = plane_view(pout, c)
                rr = off // Wo
                nc.vector.tensor_scalar(
                    v[:, 1 + rr:1 + rr + n // Wo, 1:1 + Wo],
                    s2[:, 0:n].rearrange("p (r c) -> p r c", c=Wo),
                    1.0, None, ALU.subtract)


def fix_overlaps(nc, plane_view, pl):
    """For half-split planes copy the two boundary rows into the opposite
    half's halo positions (cross-partition, so via DMA)."""
    h0, pitch, rows = pl['h0'], pl['pitch'], pl['rows']
    v0, v1 = plane_view(pl, 0), plane_view(pl, 1)
    # image row h0-1: primary = half0 buf row h0 ; -> half1 buf row 0
    nc.sync.dma_start(v1[:, 0:1, :], v0[:, h0:h0 + 1, :])
    # image row h0: primary = half1 buf row 1 ; -> half0 buf row h0+1
    nc.sync.dma_start(v0[:, h0 + 1:h0 + 2, :], v1[:, 1:2, :])


# ------------------------------------------------------------------ driver --

def kernel(x, params):
    x = np.asarray(x, np.float32)
    for (w, b, gmm, bt) in params:
        assert np.all(np.asarray(gmm) == 1.0) and np.all(np.asarray(bt) == 0.0), \
            "kernel assumes gamma=1, beta=0"
    H0 = x.shape[2]
    if 'nc' not in _CACHE:
        _CACHE['nc'] = build_program(H0)
    nc = _CACHE['nc']
    ws = prep_weights(params, H0)
    in_maps = []
    for i in range(N_CORES):
        m = {'im2col0': prep_im2col(x[i], H0)}
        m.update(ws)
        in_maps.append(m)
    res = run_bass_kernel_spmd(nc, in_maps, core_ids=list(range(N_CORES)))
    _CACHE['last_result'] = res
    Hf = make_geom(H0)[10]['Hout']
    out = np.stack([res.results[i]['out'].reshape(11, Hf, Hf)
                    for i in range(N_CORES)])
    return out.astype(np.float32)


if __name__ == "__main__":
    import pickle, time
    x = np.load('/root/problem/x.npy')
    params = pickle.load(open('/root/problem/params.pkl', 'rb'))
    ref = np.load('/root/problem/ref_out.npy')
    t0 = time.time()
    out = kernel(x, params)
    print("kernel() wall", time.time() - t0)
    err = np.abs(out - ref)
    print("abs max err", err.max(),
          "rel l2", np.linalg.norm(out - ref) / np.linalg.norm(ref),
          "bad pixels", (err > 1e-3).sum())
